# revision 1
# baseline (speedup 1.0000x reference)
"""Trainium2 Bass kernel for ConsistentSelfAttentionTile.

Reference semantics: T=449 overlapping 64-token tiles; each tile attends to
352 KV tokens = 288 sampled (from a 9x replication of the tile) + the tile
itself; outputs overlap-add, then divide by overlap counts.

Algebraic collapse used here (verified to ~1e-6 rel vs the jax reference):
  * rep[:, idx, :] == tile[:, idx % 64, :], so the sampled KV tokens are tile
    rows with integer multiplicities m_t[w] = 1 + #{s : idx[t,s] % 64 == w}.
  * Per-tile Q/K/V are slices of the full-sequence projections, so all
    per-tile 64x64 score blocks are diagonal blocks of one banded 512x512
    score matrix S = Q K^T (band |i-j| <= 63).
  * With E = exp(S - rowmax), Cm[j,t] = m_t[j-t] (banded), the full
    tile-softmax + overlap-add + count-divide collapses to
        Z = E @ Cm;  W = bandmask/(counts * Z);  U = W @ Cm^T;
        out = (E * U) @ V
    i.e. three extra banded 512x512 matmuls instead of 449 gathered
    attentions.
  * bk drops exactly: it shifts each row's scores by a constant, which the
    rowmax-subtracted softmax cancels bit-for-bit.

Sharding: 8 cores = 2 batches x 4 row-chunks of 128 output rows. Each core
computes its 128 rows end-to-end from a 256-column band of the input (no
cross-core communication); host slices/pads inputs and concatenates outputs.

Precision plan: x and the three weight matrices ship as fp16 (halves the
DMA, which is the bottleneck at ~210 GB/s/core); all matmul products
accumulate in fp32 PSUM. The score/softmax chain (Q^T, K^T, S, E, Cm, W, U)
stays in float32r (~13-bit mantissa; fp16 E would underflow to subnormals
whenever a row's in-band max sits ~16 below its window max). The value path
(V, A, out-matmul) is fp16, where rounding only mixes linearly.

Per-core inputs are packed host-side into two blobs laid out exactly as
their SBUF destinations, DMA'd in priority-chained groups (2 parallel
queues per group) so compute starts after the first ~1 MB.
"""

import os
import sys

import numpy as np

try:
    import ml_dtypes
except ImportError:
    ml_dtypes = None

for _p in ("/opt/trn_rl_repo",):
    if _p not in sys.path and os.path.isdir(_p):
        sys.path.insert(0, _p)

B, N, C, W = 2, 512, 512, 64
T = N - W + 1          # 449 tiles
RCH = 128              # output rows per core
NCORES = 8
BAND = 256             # per-core j/t band width (columns [r0-64, r0+192))
KC = C // 128          # 4 contraction chunks
JC = BAND // 128       # 2 band chunks

# blob16 layout (2-byte elements per partition; fp16 except the bf16 Cm
# segments, which are bitcast views)
OFF_XT = 0                       # [128, 4, 256] fp16
OFF_WQT = OFF_XT + KC * BAND     # [128, 4, 512] fp16
OFF_MISC = OFF_WQT + KC * C      # p0 rows: bq [512] | bv [512] | ones [128]
END16_G1 = OFF_MISC + 2 * C + 128
OFF_WKT = END16_G1               # [128, 4, 512] fp16
END16_G2 = OFF_WKT + KC * C
OFF_CM = END16_G2                # [128, 2, 256] bf16 (count ints: exact)
OFF_CMT = OFF_CM + JC * BAND     # [128, 2, 256] bf16
OFF_ID16 = OFF_CMT + JC * BAND   # [128, 128] bf16 identity
END16_G3 = OFF_ID16 + 128
OFF_WVT = END16_G3               # [128, 4, 512] fp16
F16 = OFF_WVT + KC * C

# blob32 layout (fp32 elements per partition; DMA'd with DMA group 1)
OFF_MW = 0                       # [128, 2, 128]
OFF_ID = OFF_MW + JC * RCH       # [128, 128] identity (fp32r via bitcast)
F32 = OFF_ID + 128

_CACHE = {}


def _slim_drain_and_barrier(self, tick_clock, wait_clock):
    """Cheaper TileContext exit. Every compute op in this kernel feeds the
    output DMA, so the final drain only needs to cover DMA-queue completion
    (not the full 27-proc global clock, whose multi-wait split costs an
    ~10us EVSEM butterfly). Engines are then synced with one sem-only
    barrier and the semaphores reset for NEFF re-executability."""
    from concourse.vector_clock import ScopedClock, VectorClock
    from concourse.tile_scheduler import dmasw_start_idx, N_PROCS

    g = tick_clock.global_clock
    dma_clock = VectorClock()
    for idx in range(dmasw_start_idx, N_PROCS):
        t = g.peek_next(idx) - 1
        if t > 0:
            dma_clock.require_at_least(idx, t)
    drain_inst = self.nc.sync.drain()
    wait_clock.add_sem_waits(drain_inst.ins, ScopedClock({None: dma_clock}))
    self.nc.all_engine_barrier(sem_only=True)
    popped = self.nc._tile_sem_poison_stack.pop()
    assert popped is self._sem_poison
    self.nc.clear_and_free_semaphores(list(self.sems.allocated().values()))


def _build_program():
    import concourse.bacc as bacc
    import concourse.mybir as mybir
    import concourse.tile as tile

    fp32 = mybir.dt.float32
    fp16 = mybir.dt.float16
    # Bass's preamble ends with a full all-engine barrier (drains + EVSEM,
    # ~3-5us with the PE's first-IRAM-block stall). Our kernel never reads
    # the preamble's const APs and all real cross-engine deps are Tile
    # semaphores, so skip it: engines start independently and the input DMA
    # issues ~5us earlier.
    orig_aeb = bacc.Bacc.all_engine_barrier

    def _noop_aeb(self, *, sem_only=False):
        return None

    bacc.Bacc.all_engine_barrier = _noop_aeb
    try:
        nc = bacc.Bacc("TRN2", target_bir_lowering=False, debug=False)
    finally:
        bacc.Bacc.all_engine_barrier = orig_aeb

    b16_d = nc.declare_dram_parameter("blob16", [128, F16], fp16, isOutput=False)
    b32_d = nc.declare_dram_parameter("blob32", [128, F32], fp32, isOutput=False)
    out_d = nc.declare_dram_parameter("out", [RCH, C], fp32, isOutput=True)

    orig_dab = tile.TileContext._drain_and_barrier
    tile.TileContext._drain_and_barrier = _slim_drain_and_barrier
    try:
        _emit_body(nc, tile, mybir, b16_d, b32_d, out_d)
    finally:
        tile.TileContext._drain_and_barrier = orig_dab

    nc.compile()
    return nc


def _emit_body(nc, tile, mybir, b16_d, b32_d, out_d):
    from concourse.tile_rust import add_dep_helper

    fp32 = mybir.dt.float32
    fp32r = mybir.dt.float32r
    fp16 = mybir.dt.float16

    with tile.TileContext(nc) as tc:
        with (
            tc.tile_pool(name="consts", bufs=1) as consts,
            tc.tile_pool(name="work", bufs=1) as work,
            tc.tile_pool(name="psum", bufs=1, space="PSUM") as psum,
        ):
            b16 = consts.tile([128, F16], fp16)
            b32 = consts.tile([128, F32], fp32r)
            # Priority-chained DMA groups, 3 parallel queues per group (a
            # single HWDGE queue tops out ~200 GB/s), issued alternately
            # from the two HWDGE-capable engines (sync, scalar) since each
            # PSEUDO_DMA issue costs ~0.6us of engine time. Chaining is one
            # dep per piece (index-matched) to bound the evsem-split cost.
            groups = [
                [(b16, b16_d[:], 0, END16_G1, 3),
                 (b32, b32_d[:].bitcast(fp32r), 0, F32, 1)],
                [(b16, b16_d[:], END16_G1, END16_G2, 2)],
                [(b16, b16_d[:], END16_G2, END16_G3, 1)],
                [(b16, b16_d[:], END16_G3, F16, 2)],
            ]
            issuers = [nc.sync, nc.scalar]
            prev_group = []
            n_issued = 0
            for group in groups:
                cur_group = []
                for dst, src, lo, hi, npc in group:
                    cuts = [lo + (hi - lo) * i // npc
                            for i in range(npc + 1)]
                    for a, b in zip(cuts, cuts[1:]):
                        if a == b:
                            continue
                        eng = issuers[n_issued % len(issuers)]
                        n_issued += 1
                        d = eng.dma_start(out=dst[:, a:b], in_=src[:, a:b])
                        if prev_group:
                            add_dep_helper(d.ins, prev_group[0].ins, True,
                                           "input DMA priority chain")
                        cur_group.append(d)
                prev_group = cur_group

            xt_sb = b16[:, OFF_XT:OFF_XT + KC * BAND].rearrange(
                "p (k j) -> p k j", k=KC)
            wqt_sb = b16[:, OFF_WQT:OFF_WQT + KC * C].rearrange(
                "p (k j) -> p k j", k=KC)
            wkt_sb = b16[:, OFF_WKT:OFF_WKT + KC * C].rearrange(
                "p (k j) -> p k j", k=KC)
            wvt_sb = b16[:, OFF_WVT:OFF_WVT + KC * C].rearrange(
                "p (k j) -> p k j", k=KC)
            bqr_sb = b16[0:1, OFF_MISC:OFF_MISC + C]
            bvr_sb = b16[0:1, OFF_MISC + C:OFF_MISC + 2 * C]
            ones1 = b16[0:1, OFF_MISC + 2 * C:OFF_MISC + 2 * C + 128]
            bf16 = mybir.dt.bfloat16
            cm_sb = b16[:, OFF_CM:OFF_CM + JC * BAND].bitcast(bf16).rearrange(
                "p (k t) -> p k t", k=JC)
            cmt_sb = b16[:, OFF_CMT:OFF_CMT + JC * BAND].bitcast(
                bf16).rearrange("p (k j) -> p k j", k=JC)
            mw_sb = b32[:, OFF_MW:OFF_MW + JC * RCH].bitcast(
                fp32).rearrange("p (k r) -> p k r", k=JC)
            ident = b32[:, OFF_ID:OFF_ID + 128]
            ident16 = b16[:, OFF_ID16:OFF_ID16 + 128].bitcast(bf16)

            # ---- projections (fp16 inputs, fp32 PSUM accumulation) ----
            # Q rows [r 128, c 512] (+bq via rank-1 ones matmul), then
            # transpose to QT chunks [c 128, r 128] in fp32r
            ps_qrow = psum.tile([128, C], fp32, tag="ps_big", bufs=2)
            for k in range(KC):
                nc.tensor.matmul(
                    ps_qrow,
                    lhsT=xt_sb[:, k, 64:64 + RCH],
                    rhs=wqt_sb[:, k, :],
                    start=(k == 0),
                    stop=False,
                )
            nc.tensor.matmul(
                ps_qrow, lhsT=ones1, rhs=bqr_sb, start=False, stop=True,
            )
            q_sb = work.tile([128, C], fp32r)
            nc.vector.tensor_copy(out=q_sb, in_=ps_qrow)
            qt_sb = work.tile([128, KC, RCH], fp32r)
            for m in range(KC):
                ps_t = psum.tile([128, RCH], fp32r, tag="ps_t", bufs=2)
                nc.tensor.transpose(
                    ps_t, q_sb[:, m * 128:(m + 1) * 128], ident
                )
                nc.vector.tensor_copy(out=qt_sb[:, m, :], in_=ps_t)

            # KT[m][c_out 128, j 256]  (bk dropped: softmax-invariant)
            kt_sb = work.tile([128, KC, BAND], fp32r)
            for m in range(KC):
                ps_k = psum.tile([128, BAND], fp32, tag="ps_k", bufs=1)
                for k in range(KC):
                    nc.tensor.matmul(
                        ps_k,
                        lhsT=wkt_sb[:, k, m * 128:(m + 1) * 128],
                        rhs=xt_sb[:, k, :],
                        start=(k == 0),
                        stop=(k == KC - 1),
                    )
                nc.vector.tensor_copy(out=kt_sb[:, m, :], in_=ps_k)

            # V[jc][j 128, c 512] (+bv via rank-1 ones matmul), fp16
            v_sb = work.tile([128, JC, C], fp16)
            for jc in range(JC):
                ps_v = psum.tile([128, C], fp32, tag="ps_big", bufs=2)
                for k in range(KC):
                    nc.tensor.matmul(
                        ps_v,
                        lhsT=xt_sb[:, k, jc * 128:(jc + 1) * 128],
                        rhs=wvt_sb[:, k, :],
                        start=(k == 0),
                        stop=False,
                    )
                nc.tensor.matmul(
                    ps_v, lhsT=ones1, rhs=bvr_sb, start=False, stop=True,
                )
                nc.vector.tensor_copy(out=v_sb[:, jc, :], in_=ps_v)

            # ---- scores and softmax numerator (fp32r) ----
            ps_s = psum.tile([128, BAND], fp32, tag="ps_s", bufs=1)
            for k in range(KC):
                nc.tensor.matmul(
                    ps_s,
                    lhsT=qt_sb[:, k, :],
                    rhs=kt_sb[:, k, :],
                    start=(k == 0),
                    stop=(k == KC - 1),
                )
            negmax = work.tile([128, 1], fp32)
            nc.vector.reduce_max(
                negmax, ps_s, axis=mybir.AxisListType.X, negate=True
            )
            e_sb = work.tile([128, BAND], bf16)
            nc.scalar.activation(
                out=e_sb, in_=ps_s,
                func=mybir.ActivationFunctionType.Exp,
                bias=negmax, scale=1.0,
            )

            # E^T chunks [j 128, r 128]
            et_sb = work.tile([128, JC, RCH], bf16)
            for jc in range(JC):
                ps_e = psum.tile([128, RCH], bf16, tag="ps_t", bufs=2)
                nc.tensor.transpose(
                    ps_e, e_sb[:, jc * 128:(jc + 1) * 128], ident16
                )
                nc.vector.tensor_copy(out=et_sb[:, jc, :], in_=ps_e)

            # Z'[t 128, r 128] = sum_j Cm[j,t] E'[j,r];  W' = maskw / Z'
            w_sb = work.tile([128, JC, RCH], bf16)
            for tch in range(JC):
                ps_z = psum.tile([128, RCH], fp32, tag="ps_zu", bufs=2)
                for jc in range(JC):
                    nc.tensor.matmul(
                        ps_z,
                        lhsT=cm_sb[:, jc, tch * 128:(tch + 1) * 128],
                        rhs=et_sb[:, jc, :],
                        start=(jc == 0),
                        stop=(jc == JC - 1),
                    )
                rz = work.tile([128, RCH], fp32, tag="rz", bufs=2)
                nc.vector.reciprocal(out=rz, in_=ps_z)
                nc.vector.tensor_mul(
                    w_sb[:, tch, :], rz, mw_sb[:, tch, :]
                )

            # U'[j 128, r 128] = sum_t Cm^T[t,j] W'[t,r];  A' = E' * U'
            a_sb = work.tile([128, JC, RCH], fp16)
            for jc in range(JC):
                ps_u = psum.tile([128, RCH], fp32, tag="ps_zu", bufs=2)
                for tch in range(JC):
                    nc.tensor.matmul(
                        ps_u,
                        lhsT=cmt_sb[:, tch, jc * 128:(jc + 1) * 128],
                        rhs=w_sb[:, tch, :],
                        start=(tch == 0),
                        stop=(tch == JC - 1),
                    )
                nc.vector.tensor_mul(
                    a_sb[:, jc, :], ps_u, et_sb[:, jc, :]
                )

            # out rows [r 128, c 512] = sum_j A'[j,r]^T V[j,c]  (fp16)
            ps_o = psum.tile([128, C], fp32, tag="ps_big", bufs=2)
            for jc in range(JC):
                nc.tensor.matmul(
                    ps_o,
                    lhsT=a_sb[:, jc, :],
                    rhs=v_sb[:, jc, :],
                    start=(jc == 0),
                    stop=(jc == JC - 1),
                )
            o_sb = work.tile([128, C], fp32)
            nc.vector.tensor_copy(out=o_sb, in_=ps_o)
            nc.sync.dma_start(out=out_d[:], in_=o_sb)


def _pack128(arr):
    """[n*128, f] row-chunked -> [128, n*f] (chunk-major along free axis)."""
    n = arr.shape[0] // 128
    return np.ascontiguousarray(
        arr.reshape(n, 128, -1).transpose(1, 0, 2).reshape(128, -1)
    )


def _host_prep(image_features, Wq, bq, Wk, bk, Wv, bv, sample_idx):
    """Build the 8 per-core input blobs (pure index/layout work)."""
    x = np.asarray(image_features, np.float32)
    sample_idx = np.asarray(sample_idx)

    # per-tile multiplicities -> banded count matrix Cm[j, t] = m_t[j - t]
    mod = (sample_idx % W).astype(np.int64)                  # [T, S]
    m = np.zeros((T, W), np.float32)
    np.add.at(m, (np.arange(T)[:, None], mod), 1.0)
    m += 1.0
    Cm = np.zeros((N, N), np.float32)
    rows = np.arange(T)
    for w in range(W):
        Cm[rows + w, rows] = m[:, w]

    pos = np.arange(N)
    counts = (np.minimum(pos, N - W) - np.maximum(pos - W + 1, 0) + 1)

    # padded versions for uniform band slicing
    XTp = np.zeros((B, C, N + 2 * 64), np.float16)
    for b in range(B):
        XTp[b, :, 64:64 + N] = x[b].T.astype(np.float16)
    Cmp = np.zeros((N + 2 * 64, N + 2 * 64), np.float32)
    Cmp[64:64 + N, 64:64 + N] = Cm

    wqt_p = _pack128(np.asarray(Wq, np.float32).T.astype(np.float16))
    wkt_p = _pack128(np.asarray(Wk, np.float32).T.astype(np.float16))
    wvt_p = _pack128(np.asarray(Wv, np.float32).T.astype(np.float16))

    in_maps = []
    for core in range(NCORES):
        b, rc = divmod(core, NCORES // B)
        r0 = rc * RCH
        xt = XTp[b, :, r0:r0 + BAND]
        cm = np.ascontiguousarray(Cmp[r0:r0 + BAND, r0:r0 + BAND])
        # all-zero columns (padded t) would give Z=0 -> 1/0*mask = NaN on
        # device; a diagonal 1 keeps Z finite there and is masked out of W
        zero_cols = ~cm.any(axis=0)
        cm[zero_cols, zero_cols] = 1.0
        tl = np.arange(BAND)
        rl = np.arange(RCH)
        tg = r0 - 64 + tl
        rg = r0 + rl
        d = rg[None, :] - tg[:, None]
        valid = (d >= 0) & (d <= W - 1) & (tg[:, None] >= 0) & (tg[:, None] <= T - 1)
        maskw = np.where(
            valid, 1.0 / counts[rg][None, :], 0.0
        ).astype(np.float32)

        b16 = np.zeros((128, F16), np.float16)
        b16[:, OFF_XT:OFF_XT + KC * BAND] = _pack128(xt)
        b16[:, OFF_WQT:OFF_WQT + KC * C] = wqt_p
        b16[:, OFF_WKT:OFF_WKT + KC * C] = wkt_p
        b16[:, OFF_WVT:OFF_WVT + KC * C] = wvt_p
        b16[0, OFF_MISC:OFF_MISC + C] = np.asarray(bq, np.float32)
        b16[0, OFF_MISC + C:OFF_MISC + 2 * C] = np.asarray(bv, np.float32)
        b16[0, OFF_MISC + 2 * C:OFF_MISC + 2 * C + 128] = 1.0
        # Cm segments carry bf16 bits (count ints are exact in bf16);
        # written through a uint16 view of the fp16 buffer
        b16v = b16.view(np.uint16)
        b16v[:, OFF_CM:OFF_CM + JC * BAND] = _pack128(
            cm.astype(ml_dtypes.bfloat16)).view(np.uint16)
        b16v[:, OFF_CMT:OFF_CMT + JC * BAND] = _pack128(
            np.ascontiguousarray(cm.T).astype(ml_dtypes.bfloat16)
        ).view(np.uint16)

        b16v[:, OFF_ID16:OFF_ID16 + 128] = np.eye(
            128, dtype=ml_dtypes.bfloat16).view(np.uint16)

        b32 = np.zeros((128, F32), np.float32)
        b32[:, OFF_MW:OFF_MW + JC * RCH] = _pack128(maskw)
        b32[:, OFF_ID:OFF_ID + 128] = np.eye(128, dtype=np.float32)
        in_maps.append({"blob16": b16, "blob32": b32})
    return in_maps


def run_on_cores(in_maps, trace=False, trace_cores=None):
    from concourse.bass_utils import run_bass_kernel_spmd

    if "nc" not in _CACHE:
        _CACHE["nc"] = _build_program()
    nc = _CACHE["nc"]
    return run_bass_kernel_spmd(
        nc, in_maps, list(range(NCORES)), trace=trace,
        trace_cores=(trace_cores or [0]) if trace else None,
    )


def kernel(image_features, Wq, bq, Wk, bk, Wv, bv, sample_idx):
    in_maps = _host_prep(image_features, Wq, bq, Wk, bk, Wv, bv, sample_idx)
    res = run_on_cores(in_maps, trace=False)
    out = np.empty((B, N, C), np.float32)
    for core in range(NCORES):
        b, rc = divmod(core, NCORES // B)
        out[b, rc * RCH:(rc + 1) * RCH, :] = res.results[core]["out"]
    return out



# revision 16
# speedup vs baseline: 1.2217x; 1.2217x over previous
"""Trainium2 Bass kernel for ConsistentSelfAttentionTile.

Reference semantics: T=449 overlapping 64-token tiles; each tile attends to
352 KV tokens = 288 sampled (from a 9x replication of the tile) + the tile
itself; outputs overlap-add, then divide by overlap counts.

Algebraic collapse (same as the verified baseline):
  * rep[:, idx, :] == tile[:, idx % 64, :], so sampled KV tokens are tile
    rows with multiplicities m_t[w] = 1 + #{s : idx[t,s] % 64 == w}.
  * All per-tile 64x64 score blocks are diagonal blocks of one banded
    512x512 score matrix S = Q K^T (band |i-j| <= 63).
  * With E^T = exp(S^T + kb - 40), Cm[j,t] = m_t[j-t] (banded), the
    tile-softmax + overlap-add + count-divide collapses to
        Z = Cm^T E^T; W = mask/(counts * Z); U = Cm W; out = (E^T*U)^T V
    The constant -40 shift (vs a per-row max) cancels exactly; kb[j] =
    K[j].bq carries the Q bias (host-precomputed as x_band @ (Wk^T bq),
    exact), so Q/K projections are bias-free on device.
  * bk drops exactly (softmax-invariant); bv adds at the end as a rank-1
    +1*bv (attention rows sum to exactly 1), so V is bias-free too.

v2 vs baseline (37.9us): scores computed TRANSPOSED (st = K Q^T) so the
exp writes E^T directly - no reduce_max, no PE transposes, no identity
matrices. Input DMA is 9 unchained FIFO pieces across the two HWDGE rings
(sync+scalar), ordered by need-time, so the ~2.2 MB lands in ~7us instead
of trickling over 15. Dummy matmuls at window start warm the PE HAM clock
gate before real work arrives. PSUM drains split across ACT+DVE;
1/Z via reciprocal_approx_fast; output DMA'd in 2 halves on 2 rings.

Sharding: 8 cores = 2 batches x 4 row-chunks of 128 output rows, each core
fully independent on a 256-column band of its batch's sequence.
"""

import os
import sys

import numpy as np

try:
    import ml_dtypes
except ImportError:
    ml_dtypes = None

for _p in ("/opt/trn_rl_repo",):
    if _p not in sys.path and os.path.isdir(_p):
        sys.path.insert(0, _p)

B, N, C, W = 2, 512, 512, 64
T = N - W + 1          # 449 tiles
RCH = 128              # output rows per core
NCORES = 8
BAND = 256             # per-core j/t band width (columns [r0-64, r0+192))
KC = C // 128          # 4 contraction chunks
JC = BAND // 128       # 2 band chunks
ESHIFT = -40.0         # constant exp shift (cancels exactly; keeps the
                       # activation-table inputs in the proven-negative range)

# blob layout (fp16 columns; cm/cmt bf16 and kb fp32 are bitcast views)
OFF_XT = 0                       # [128, 4, 256] fp16 x^T band
OFF_KB = OFF_XT + KC * BAND      # [128, 2] fp32 exp bias kb[j]+ESHIFT
OFF_WQT = OFF_KB + 4             # [128, 4, 512] fp16
OFF_CM = OFF_WQT + KC * C        # [128, 2, 256] bf16 Cm[j, t]
OFF_CMT = OFF_CM + JC * BAND     # [128, 2, 256] bf16 Cm^T[t, j]
OFF_MW = OFF_CMT + JC * BAND     # [128, 2, 128] fp16 mask/counts
OFF_WKT = OFF_MW + JC * RCH      # [128, 4, 512] fp16
OFF_WVT = OFF_WKT + KC * C       # [128, 4, 512] fp16
F16 = OFF_WVT + KC * C

_CACHE = {}


def _slim_drain_and_barrier(self, tick_clock, wait_clock):
    """Cheaper TileContext exit: final drain covers only DMA-queue
    completion, then one sem-only barrier + semaphore reset."""
    from concourse.vector_clock import ScopedClock, VectorClock
    from concourse.tile_scheduler import dmasw_start_idx, N_PROCS

    g = tick_clock.global_clock
    dma_clock = VectorClock()
    for idx in range(dmasw_start_idx, N_PROCS):
        t = g.peek_next(idx) - 1
        if t > 0:
            dma_clock.require_at_least(idx, t)
    drain_inst = self.nc.sync.drain()
    wait_clock.add_sem_waits(drain_inst.ins, ScopedClock({None: dma_clock}))
    self.nc.all_engine_barrier(sem_only=True)
    popped = self.nc._tile_sem_poison_stack.pop()
    assert popped is self._sem_poison
    self.nc.clear_and_free_semaphores(list(self.sems.allocated().values()))


def _build_program():
    import concourse.bacc as bacc
    import concourse.mybir as mybir
    import concourse.tile as tile

    fp32 = mybir.dt.float32
    fp16 = mybir.dt.float16
    # Skip Bass's preamble all-engine barrier (we never read the preamble
    # const APs via cross-engine paths; Tile sems carry all real deps), so
    # the input DMA issues ~5us earlier.
    orig_aeb = bacc.Bacc.all_engine_barrier

    def _noop_aeb(self, *, sem_only=False):
        return None

    bacc.Bacc.all_engine_barrier = _noop_aeb
    try:
        nc = bacc.Bacc("TRN2", target_bir_lowering=False, debug=False)
    finally:
        bacc.Bacc.all_engine_barrier = orig_aeb

    blob_d = nc.declare_dram_parameter("blob", [128, F16], fp16, isOutput=False)
    bv_d = nc.declare_dram_parameter("bvrow", [1, C], fp16, isOutput=False)
    out_d = nc.declare_dram_parameter("out", [RCH, C], fp32, isOutput=True)

    orig_dab = tile.TileContext._drain_and_barrier
    tile.TileContext._drain_and_barrier = _slim_drain_and_barrier
    try:
        _emit_body(nc, tile, mybir, blob_d, bv_d, out_d)
    finally:
        tile.TileContext._drain_and_barrier = orig_dab

    nc.compile()
    return nc


def _emit_body(nc, tile, mybir, blob_d, bv_d, out_d):
    fp32 = mybir.dt.float32
    fp16 = mybir.dt.float16
    bf16 = mybir.dt.bfloat16
    AFT = mybir.ActivationFunctionType

    with tile.TileContext(nc) as tc:
        with (
            tc.tile_pool(name="consts", bufs=1) as consts,
            tc.tile_pool(name="work", bufs=1) as work,
            tc.tile_pool(name="psum", bufs=1, space="PSUM") as psum,
        ):
            blob = consts.tile([128, F16], fp16)
            bv_sb = consts.tile([1, C], fp16)

            # scratch for PE warm-up + the rank-1 ones operand
            dum_src = work.tile([128, 256], fp16)
            ones_sb = work.tile([1, 128], fp16)
            nc.gpsimd.memset(dum_src, 0.0)
            nc.gpsimd.memset(ones_sb, 1.0)

            # ---- input DMA: unchained FIFO pieces on the two HWDGE rings,
            # ordered by need-time (each ring drains in instruction order;
            # the 16 SDMA engines round-robin the two rings' packets).
            sync_pieces = [
                (OFF_XT, OFF_WQT),               # x + kb
                (OFF_WQT, OFF_CM),               # wq (QT is m-sequential)
                (OFF_WVT, OFF_WVT + 2 * C),      # wv k0,k1 (arrives last here)
            ]
            scalar_pieces = [
                (OFF_WKT, OFF_WKT + 2 * C),      # wk k0,k1
                (OFF_WKT + 2 * C, OFF_WVT),      # wk k2,k3
                (OFF_WVT + 2 * C, F16),          # wv k2,k3
                (OFF_CM, OFF_WKT),               # cm + cmt + mw (shortest tail)
            ]
            nc.scalar.dma_start(out=bv_sb, in_=bv_d[:])
            for lo, hi in sync_pieces:
                nc.sync.dma_start(out=blob[:, lo:hi], in_=blob_d[:, lo:hi])
            for lo, hi in scalar_pieces:
                nc.scalar.dma_start(out=blob[:, lo:hi], in_=blob_d[:, lo:hi])

            xt = blob[:, OFF_XT:OFF_XT + KC * BAND].rearrange(
                "p (k j) -> p k j", k=KC)
            wqt = blob[:, OFF_WQT:OFF_WQT + KC * C].rearrange(
                "p (k j) -> p k j", k=KC)
            wkt = blob[:, OFF_WKT:OFF_WKT + KC * C].rearrange(
                "p (k j) -> p k j", k=KC)
            wvt = blob[:, OFF_WVT:OFF_WVT + KC * C].rearrange(
                "p (k j) -> p k j", k=KC)
            kb = blob[:, OFF_KB:OFF_KB + 4].bitcast(fp32)
            cm = blob[:, OFF_CM:OFF_CM + JC * BAND].bitcast(bf16).rearrange(
                "p (k t) -> p k t", k=JC)
            cmt = blob[:, OFF_CMT:OFF_CMT + JC * BAND].bitcast(bf16).rearrange(
                "p (k j) -> p k j", k=JC)
            mw = blob[:, OFF_MW:OFF_MW + JC * RCH].rearrange(
                "p (k r) -> p k r", k=JC)

            # ---- PE warm-up: dependency-free matmuls burn the cold HAM
            # window (PE runs at 1.2 GHz until ~3.4us of activity).
            ps_dum = psum.tile([128, 256], fp32, tag="ps_o", bufs=2)
            for _ in range(10):
                nc.tensor.matmul(
                    ps_dum, lhsT=dum_src[:, 0:128], rhs=dum_src,
                    start=True, stop=True,
                )

            # PSUM is 8 banks x 2KB: KT 2 banks (2 m-chunks each), QT 1
            # bank (4 chunks), V 2, st/Z/U rotate through 1, out 2
            # (shared with the warm-up scratch).
            ps_kt = [psum.tile([128, 2 * BAND], fp32, tag="ps_k", bufs=2,
                               name=f"ps_kt{h}")
                     for h in range(2)]

            # ---- KT[m][c_out 128, j 256] = Wk x^T. PSUM constraint: only
            # ONE open accumulation group per bank (start=True clobbers
            # the sibling chunk's has_written state), so bankA runs m0
            # then m1, bankB m2 then m3; m0/m2 pipeline across wk halves.
            kt_sb = work.tile([128, KC, BAND], fp16)

            def kt_mm(m, k, start, stop):
                nc.tensor.matmul(
                    ps_kt[m // 2][:, (m % 2) * BAND:(m % 2 + 1) * BAND],
                    lhsT=wkt[:, k, m * 128:(m + 1) * 128],
                    rhs=xt[:, k, :],
                    start=start, stop=stop, skip_group_check=True,
                )

            def kt_drain(m):
                src = ps_kt[m // 2][:, (m % 2) * BAND:(m % 2 + 1) * BAND]
                if m % 2 == 0:
                    nc.scalar.copy(out=kt_sb[:, m, :], in_=src)
                else:
                    nc.vector.tensor_copy(out=kt_sb[:, m, :], in_=src)

            for m in (0, 2):                       # wk k0,k1 available
                for k in (0, 1):
                    kt_mm(m, k, start=(k == 0), stop=False)
            for m in (0, 2):                       # wk k2,k3 available
                for k in (2, 3):
                    kt_mm(m, k, start=False, stop=(k == 3))
                kt_drain(m)
            for m in (1, 3):                       # banks free after m0/m2
                for k in range(KC):
                    kt_mm(m, k, start=(k == 0), stop=(k == KC - 1))
                kt_drain(m)

            # ---- QT[m][c_out 128, r 128] = Wq x_q^T (no bias: bq rides
            # the exp bias via kb). One bank: strictly m-sequential.
            ps_qt = psum.tile([128, KC * RCH], fp32, tag="ps_q", bufs=1)
            qt_sb = work.tile([128, KC, RCH], fp16)
            for m in range(KC):
                for k in range(KC):
                    nc.tensor.matmul(
                        ps_qt[:, m * RCH:(m + 1) * RCH],
                        lhsT=wqt[:, k, m * 128:(m + 1) * 128],
                        rhs=xt[:, k, 64:64 + RCH],
                        start=(k == 0),
                        stop=(k == KC - 1),
                        skip_group_check=True,
                    )
                src = ps_qt[:, m * RCH:(m + 1) * RCH]
                if m % 2 == 0:
                    nc.scalar.copy(out=qt_sb[:, m, :], in_=src)
                else:
                    nc.vector.tensor_copy(out=qt_sb[:, m, :], in_=src)

            # ---- V[jc][j 128, c 512] = x_band Wv^T (no bias: bv adds at
            # the output). k-order 2,3,0,1 matches wv DMA arrival.
            ps_v = [psum.tile([128, C], fp32, tag="ps_v", bufs=2,
                              name=f"ps_v{jc}")
                    for jc in range(JC)]
            for k in (2, 3):
                for jc in range(JC):
                    nc.tensor.matmul(
                        ps_v[jc],
                        lhsT=xt[:, k, jc * 128:(jc + 1) * 128],
                        rhs=wvt[:, k, :],
                        start=(k == 2),
                        stop=False,
                    )

            # ---- st[jc][j 128, r 128] = K Q^T (scores TRANSPOSED) then
            # E^T = exp(st + kb[j] - 40) straight to SBUF bf16: no rowmax
            # (the shift is constant per column), no E transpose.
            et_sb = work.tile([128, JC, RCH], bf16)
            ps_st = psum.tile([128, JC * RCH], fp32, tag="ps_x", bufs=1)
            for jc in range(JC):
                for m in range(KC):
                    nc.tensor.matmul(
                        ps_st[:, jc * RCH:(jc + 1) * RCH],
                        lhsT=kt_sb[:, m, jc * 128:(jc + 1) * 128],
                        rhs=qt_sb[:, m, :],
                        start=(m == 0),
                        stop=(m == KC - 1),
                        skip_group_check=True,
                    )
                nc.scalar.activation(
                    out=et_sb[:, jc, :], in_=ps_st[:, jc * RCH:(jc + 1) * RCH],
                    func=AFT.Exp, bias=kb[:, jc:jc + 1], scale=1.0,
                )

            # ---- V k0,k1 (wv01 is the last sync-ring piece)
            for k in (0, 1):
                for jc in range(JC):
                    nc.tensor.matmul(
                        ps_v[jc],
                        lhsT=xt[:, k, jc * 128:(jc + 1) * 128],
                        rhs=wvt[:, k, :],
                        start=False,
                        stop=(k == 1),
                    )

            # ---- Z[tch][t 128, r 128] = Cm^T E^T;  W = mw / Z
            w_sb = work.tile([128, JC, RCH], bf16)
            rz = work.tile([128, JC, RCH], fp32)
            ps_z = psum.tile([128, JC * RCH], fp32, tag="ps_x", bufs=1)
            for tch in range(JC):
                for jc in range(JC):
                    nc.tensor.matmul(
                        ps_z[:, tch * RCH:(tch + 1) * RCH],
                        lhsT=cm[:, jc, tch * 128:(tch + 1) * 128],
                        rhs=et_sb[:, jc, :],
                        start=(jc == 0),
                        stop=(jc == JC - 1),
                        skip_group_check=True,
                    )
                nc.vector.reciprocal_approx_fast(
                    out=rz[:, tch, :], in_=ps_z[:, tch * RCH:(tch + 1) * RCH])
                nc.vector.tensor_mul(
                    w_sb[:, tch, :], rz[:, tch, :], mw[:, tch, :]
                )

            # ---- V drains (4 half-drains split DVE/ACT)
            v_sb = work.tile([128, JC, C], fp16)
            for jc in range(JC):
                nc.vector.tensor_copy(
                    out=v_sb[:, jc, 0:256], in_=ps_v[jc][:, 0:256])
                nc.scalar.copy(
                    out=v_sb[:, jc, 256:512], in_=ps_v[jc][:, 256:512])

            # ---- U[jc][j 128, r 128] = Cm W;  A = E^T * U
            a_sb = work.tile([128, JC, RCH], fp16)
            ps_u = psum.tile([128, JC * RCH], fp32, tag="ps_x", bufs=1)
            for jc in range(JC):
                for tch in range(JC):
                    nc.tensor.matmul(
                        ps_u[:, jc * RCH:(jc + 1) * RCH],
                        lhsT=cmt[:, tch, jc * 128:(jc + 1) * 128],
                        rhs=w_sb[:, tch, :],
                        start=(tch == 0),
                        stop=(tch == JC - 1),
                        skip_group_check=True,
                    )
                nc.vector.tensor_mul(
                    a_sb[:, jc, :], ps_u[:, jc * RCH:(jc + 1) * RCH],
                    et_sb[:, jc, :]
                )

            # ---- out[r 128, c 512] = A^T V + 1 (x) bv, in 2 column
            # halves so the first DMA overlaps the second half's drain.
            o_sb = work.tile([128, C], fp32)
            for ch in range(2):
                cs = ch * (C // 2)
                ps_o = psum.tile([128, C // 2], fp32, tag="ps_o", bufs=2,
                                 name=f"ps_o{ch}")
                nc.tensor.matmul(
                    ps_o, lhsT=ones_sb, rhs=bv_sb[:, cs:cs + C // 2],
                    start=True, stop=False,
                )
                for jc in range(JC):
                    nc.tensor.matmul(
                        ps_o,
                        lhsT=a_sb[:, jc, :],
                        rhs=v_sb[:, jc, cs:cs + C // 2],
                        start=False,
                        stop=(jc == JC - 1),
                    )
                if ch == 0:
                    nc.vector.tensor_copy(
                        out=o_sb[:, cs:cs + C // 2], in_=ps_o)
                    nc.sync.dma_start(
                        out=out_d[:, cs:cs + C // 2],
                        in_=o_sb[:, cs:cs + C // 2])
                else:
                    nc.scalar.copy(out=o_sb[:, cs:cs + C // 2], in_=ps_o)
                    nc.scalar.dma_start(
                        out=out_d[:, cs:cs + C // 2],
                        in_=o_sb[:, cs:cs + C // 2])


def _pack128(arr):
    """[n*128, f] row-chunked -> [128, n*f] (chunk-major along free axis)."""
    n = arr.shape[0] // 128
    return np.ascontiguousarray(
        arr.reshape(n, 128, -1).transpose(1, 0, 2).reshape(128, -1)
    )


def _host_prep(image_features, Wq, bq, Wk, bk, Wv, bv, sample_idx):
    """Build the 8 per-core input blobs (pure index/layout work plus one
    tiny matvec kb = x_band @ (Wk^T bq) that folds bq into the exp bias)."""
    x = np.asarray(image_features, np.float32)
    sample_idx = np.asarray(sample_idx)

    # per-tile multiplicities -> banded count matrix Cm[j, t] = m_t[j - t]
    mod = (sample_idx % W).astype(np.int64)                  # [T, S]
    m = np.zeros((T, W), np.float32)
    np.add.at(m, (np.arange(T)[:, None], mod), 1.0)
    m += 1.0
    Cm = np.zeros((N, N), np.float32)
    rows = np.arange(T)
    for w in range(W):
        Cm[rows + w, rows] = m[:, w]

    pos = np.arange(N)
    counts = (np.minimum(pos, N - W) - np.maximum(pos - W + 1, 0) + 1)

    # padded versions for uniform band slicing
    XTp = np.zeros((B, C, N + 2 * 64), np.float16)
    for b in range(B):
        XTp[b, :, 64:64 + N] = x[b].T.astype(np.float16)
    Cmp = np.zeros((N + 2 * 64, N + 2 * 64), np.float32)
    Cmp[64:64 + N, 64:64 + N] = Cm

    wqt_p = _pack128(np.asarray(Wq, np.float32).T.astype(np.float16))
    wkt_p = _pack128(np.asarray(Wk, np.float32).T.astype(np.float16))
    wvt_p = _pack128(np.asarray(Wv, np.float32).T.astype(np.float16))
    bv_row = np.asarray(bv, np.float32).astype(np.float16)[None, :]
    # g = Wk^T bq; kb_full[padded j] = x_pad[j] . g  (fp16 x to match the
    # device's K = fp16(x) @ Wk^T as closely as the bias term needs)
    g = (np.asarray(Wk, np.float32).T @ np.asarray(bq, np.float32))
    kb_full = np.zeros((B, N + 2 * 64), np.float32)
    for b in range(B):
        kb_full[b] = XTp[b].astype(np.float32).T @ g

    in_maps = []
    for core in range(NCORES):
        b, rc = divmod(core, NCORES // B)
        r0 = rc * RCH
        xt = XTp[b, :, r0:r0 + BAND]
        cmb = np.ascontiguousarray(Cmp[r0:r0 + BAND, r0:r0 + BAND])
        # all-zero columns (padded t) would give Z=0 -> 1/0*mask = NaN on
        # device; a diagonal 1 keeps Z finite there and is masked out of W
        zero_cols = ~cmb.any(axis=0)
        cmb[zero_cols, zero_cols] = 1.0
        tl = np.arange(BAND)
        rl = np.arange(RCH)
        tg = r0 - 64 + tl
        rg = r0 + rl
        d = rg[None, :] - tg[:, None]
        valid = (d >= 0) & (d <= W - 1) & (tg[:, None] >= 0) & (tg[:, None] <= T - 1)
        maskw = np.where(
            valid, 1.0 / counts[rg][None, :], 0.0
        ).astype(np.float16)
        kbias = (kb_full[b, r0:r0 + BAND] + ESHIFT).astype(np.float32)

        blob = np.zeros((128, F16), np.float16)
        blob[:, OFF_XT:OFF_XT + KC * BAND] = _pack128(xt)
        blob[:, OFF_WQT:OFF_WQT + KC * C] = wqt_p
        blob[:, OFF_WKT:OFF_WKT + KC * C] = wkt_p
        blob[:, OFF_WVT:OFF_WVT + KC * C] = wvt_p
        blob[:, OFF_MW:OFF_MW + JC * RCH] = _pack128(maskw)
        blobv = blob.view(np.uint16)
        # kb as a [128, 2] fp32 bitcast (j chunk-major)
        blobv[:, OFF_KB:OFF_KB + 4] = (
            kbias.reshape(JC, 128).T.copy().view(np.uint16))
        # Cm segments carry bf16 bits (count ints are exact in bf16)
        blobv[:, OFF_CM:OFF_CM + JC * BAND] = _pack128(
            cmb.astype(ml_dtypes.bfloat16)).view(np.uint16)
        blobv[:, OFF_CMT:OFF_CMT + JC * BAND] = _pack128(
            np.ascontiguousarray(cmb.T).astype(ml_dtypes.bfloat16)
        ).view(np.uint16)
        in_maps.append({"blob": blob, "bvrow": bv_row})
    return in_maps


def run_on_cores(in_maps, trace=False, trace_cores=None):
    from concourse.bass_utils import run_bass_kernel_spmd

    if "nc" not in _CACHE:
        _CACHE["nc"] = _build_program()
    nc = _CACHE["nc"]
    return run_bass_kernel_spmd(
        nc, in_maps, list(range(NCORES)), trace=trace,
        trace_cores=(trace_cores or [0]) if trace else None,
    )


def kernel(image_features, Wq, bq, Wk, bk, Wv, bv, sample_idx):
    in_maps = _host_prep(image_features, Wq, bq, Wk, bk, Wv, bv, sample_idx)
    res = run_on_cores(in_maps, trace=False)
    out = np.empty((B, N, C), np.float32)
    for core in range(NCORES):
        b, rc = divmod(core, NCORES // B)
        out[b, rc * RCH:(rc + 1) * RCH, :] = res.results[core]["out"]
    return out


# revision 20
# speedup vs baseline: 1.2959x; 1.0607x over previous
"""Trainium2 Bass kernel for ConsistentSelfAttentionTile.

Reference semantics: T=449 overlapping 64-token tiles; each tile attends to
352 KV tokens = 288 sampled (from a 9x replication of the tile) + the tile
itself; outputs overlap-add, then divide by overlap counts.

Algebraic collapse (same as the verified baseline):
  * rep[:, idx, :] == tile[:, idx % 64, :], so sampled KV tokens are tile
    rows with multiplicities m_t[w] = 1 + #{s : idx[t,s] % 64 == w}.
  * All per-tile 64x64 score blocks are diagonal blocks of one banded
    512x512 score matrix S = Q K^T (band |i-j| <= 63).
  * With E^T = exp(S^T + kb - 40), Cm[j,t] = m_t[j-t] (banded), the
    tile-softmax + overlap-add + count-divide collapses to
        Z = Cm^T E^T; W = mask/(counts * Z); U = Cm W; out = (E^T*U)^T V
    The constant -40 shift (vs a per-row max) cancels exactly; kb[j] =
    K[j].bq carries the Q bias (host-precomputed as x_band @ (Wk^T bq),
    exact), so Q/K projections are bias-free on device.
  * bk drops exactly (softmax-invariant); bv adds on the HOST to the
    returned output (attention rows sum to exactly 1), so V is bias-free.

v4 design notes (baseline 37.9us -> v3 31.3us -> this):
  * Input DMA: 9 unchained FIFO pieces on the two HWDGE rings (sync +
    scalar), ordered by need-time; x/wq/wk ship in k-chunk halves so the
    projections start accumulating as soon as each half lands. Cm/CmT
    ship as uint8 and cast to bf16 by a GPSIMD (SWDGE) DMA on its own
    ring. Output is fp16 (host casts back) in 2 halves on 2 rings.
  * PE runs at 1.2 GHz until the HAM sees ~3.4us of CONTINUOUS activity
    (idle gaps re-throttle): dependency-free warm-up matmuls run under
    the DMA lead-in and between arrival-gated phases.
  * PSUM: one OPEN accumulation group per bank at a time (start=True
    clobbers sibling has_written). 8 banks: KT 2, QT->st->Z->U rotate 2,
    V 2, out 2 (shared with warm-up).
  * exp bias rides per-partition (kb - 40); 1/Z via the ~5x-faster
    reciprocal_approx_fast; drains split across ACT and DVE.

Sharding: 8 cores = 2 batches x 4 row-chunks of 128 output rows, each core
fully independent on a 256-column band of its batch's sequence.
"""

import os
import sys

import numpy as np

try:
    import ml_dtypes
except ImportError:
    ml_dtypes = None

for _p in ("/opt/trn_rl_repo",):
    if _p not in sys.path and os.path.isdir(_p):
        sys.path.insert(0, _p)

B, N, C, W = 2, 512, 512, 64
T = N - W + 1          # 449 tiles
RCH = 128              # output rows per core
NCORES = 8
BAND = 256             # per-core j/t band width (columns [r0-64, r0+192))
KC = C // 128          # 4 contraction chunks
JC = BAND // 128       # 2 band chunks
ESHIFT = -40.0         # constant exp shift (cancels exactly; keeps the
                       # activation-table inputs in the proven-negative range)

# blob layout (fp16 columns; kb is a [128,2] fp32 bitcast view). Order is
# the DMA piece order: sync ring x+kb | wq | wv01, scalar ring wk | wv23;
# cm/cmt (uint8) and mw ride the GPSIMD SWDGE ring.
OFF_XT = 0                       # [128, 4, 256] x^T band
OFF_KB = OFF_XT + KC * BAND      # [128, 2] fp32 exp bias kb[j]+ESHIFT
OFF_WQT = OFF_KB + 4             # [128, 4, 512]
OFF_WVT01 = OFF_WQT + KC * C     # [128, 2, 512] wv k0,k1
OFF_WKT = OFF_WVT01 + 2 * C      # [128, 4, 512]
OFF_WVT23 = OFF_WKT + KC * C     # [128, 2, 512] wv k2,k3
OFF_MW = OFF_WVT23 + 2 * C       # [128, 2, 128] fp16 mask/counts
F16 = OFF_MW + JC * RCH

NCM = 2 * JC * BAND              # cm | cmt as uint8, cast to bf16 on-chip

_CACHE = {}


def _slim_drain_and_barrier(self, tick_clock, wait_clock):
    """Cheaper TileContext exit: final drain covers only DMA-queue
    completion, then one sem-only barrier + semaphore reset."""
    from concourse.vector_clock import ScopedClock, VectorClock
    from concourse.tile_scheduler import dmasw_start_idx, N_PROCS

    g = tick_clock.global_clock
    dma_clock = VectorClock()
    for idx in range(dmasw_start_idx, N_PROCS):
        t = g.peek_next(idx) - 1
        if t > 0:
            dma_clock.require_at_least(idx, t)
    drain_inst = self.nc.sync.drain()
    wait_clock.add_sem_waits(drain_inst.ins, ScopedClock({None: dma_clock}))
    self.nc.all_engine_barrier(sem_only=True)
    popped = self.nc._tile_sem_poison_stack.pop()
    assert popped is self._sem_poison
    self.nc.clear_and_free_semaphores(list(self.sems.allocated().values()))


def _build_program():
    import concourse.bacc as bacc
    import concourse.mybir as mybir
    import concourse.tile as tile

    fp16 = mybir.dt.float16
    uint8 = mybir.dt.uint8
    # Skip Bass's preamble all-engine barrier (Tile sems carry all real
    # deps), so the input DMA issues ~5us earlier.
    orig_aeb = bacc.Bacc.all_engine_barrier

    def _noop_aeb(self, *, sem_only=False):
        return None

    bacc.Bacc.all_engine_barrier = _noop_aeb
    try:
        nc = bacc.Bacc("TRN2", target_bir_lowering=False, debug=False)
    finally:
        bacc.Bacc.all_engine_barrier = orig_aeb

    blob_d = nc.declare_dram_parameter("blob", [128, F16], fp16, isOutput=False)
    cm_d = nc.declare_dram_parameter("cmu8", [128, NCM], uint8, isOutput=False)
    out_d = nc.declare_dram_parameter("out", [RCH, C], fp16, isOutput=True)

    orig_dab = tile.TileContext._drain_and_barrier
    tile.TileContext._drain_and_barrier = _slim_drain_and_barrier
    try:
        _emit_body(nc, tile, mybir, blob_d, cm_d, out_d)
    finally:
        tile.TileContext._drain_and_barrier = orig_dab

    nc.compile()
    return nc


def _emit_body(nc, tile, mybir, blob_d, cm_d, out_d):
    fp32 = mybir.dt.float32
    fp16 = mybir.dt.float16
    bf16 = mybir.dt.bfloat16
    AFT = mybir.ActivationFunctionType

    with tile.TileContext(nc) as tc:
        with (
            tc.tile_pool(name="consts", bufs=1) as consts,
            tc.tile_pool(name="work", bufs=1) as work,
            tc.tile_pool(name="psum", bufs=1, space="PSUM") as psum,
        ):
            blob = consts.tile([128, F16], fp16)
            cmu_sb = consts.tile([128, NCM], mybir.dt.uint8)
            cmx_sb = consts.tile([128, NCM], bf16)
            dum_src = work.tile([128, 256], fp16)
            nc.gpsimd.memset(dum_src, 0.0)

            # ---- input DMA: unchained FIFO pieces; each ring drains in
            # instruction order and the SDMA engines round-robin rings.
            sync_pieces = [
                (OFF_XT, OFF_WQT),               # x + kb
                (OFF_WQT, OFF_WVT01),            # wq
                (OFF_WVT01, OFF_WKT),            # wv k0,k1
            ]
            scalar_pieces = [
                (OFF_WKT, OFF_WVT23),            # wk
                (OFF_WVT23, OFF_MW),             # wv k2,k3
            ]
            for lo, hi in sync_pieces:
                nc.sync.dma_start(out=blob[:, lo:hi], in_=blob_d[:, lo:hi])
            for lo, hi in scalar_pieces:
                nc.scalar.dma_start(out=blob[:, lo:hi], in_=blob_d[:, lo:hi])
            # cm/cmt (uint8, cast to bf16 on-chip by the otherwise-idle
            # GPSIMD) and mw ride the SWDGE ring
            nc.gpsimd.dma_start(out=cmu_sb, in_=cm_d[:])
            nc.gpsimd.dma_start(
                out=blob[:, OFF_MW:F16], in_=blob_d[:, OFF_MW:F16])
            nc.gpsimd.tensor_copy(out=cmx_sb, in_=cmu_sb)

            xt_v = blob[:, OFF_XT:OFF_XT + KC * BAND].rearrange(
                "p (k j) -> p k j", k=KC)

            def xt(k):
                return xt_v[:, k, :]

            wqt = blob[:, OFF_WQT:OFF_WQT + KC * C].rearrange(
                "p (k j) -> p k j", k=KC)
            wkt = blob[:, OFF_WKT:OFF_WKT + KC * C].rearrange(
                "p (k j) -> p k j", k=KC)
            wvt01 = blob[:, OFF_WVT01:OFF_WVT01 + 2 * C].rearrange(
                "p (k j) -> p k j", k=2)
            wvt23 = blob[:, OFF_WVT23:OFF_WVT23 + 2 * C].rearrange(
                "p (k j) -> p k j", k=2)

            def wvt(k):
                return wvt01[:, k, :] if k < 2 else wvt23[:, k - 2, :]
            kb = blob[:, OFF_KB:OFF_KB + 4].bitcast(fp32)
            mw = blob[:, OFF_MW:OFF_MW + JC * RCH].rearrange(
                "p (k r) -> p k r", k=JC)
            cm = cmx_sb[:, 0:JC * BAND].rearrange("p (k t) -> p k t", k=JC)
            cmt = cmx_sb[:, JC * BAND:NCM].rearrange("p (k j) -> p k j", k=JC)

            # ---- PE warm-up: dependency-free matmuls keep the HAM clock
            # gate fed while DMA streams (PE is 1.2 GHz until ~3.4us of
            # continuous activity; gaps re-throttle).
            ps_dum = psum.tile([128, 256], fp32, tag="ps_o", bufs=2)

            def dummies(n):
                for _ in range(n):
                    nc.tensor.matmul(
                        ps_dum, lhsT=dum_src[:, 0:128], rhs=dum_src,
                        start=True, stop=True,
                    )

            dummies(22)

            # ---- KT[m][c_out 128, j 256] = Wk x^T. One OPEN accumulation
            # group per bank: bankA m0 then m1, bankB m2 then m3; m0/m2
            # pipeline across the wk k-halves.
            ps_kt = [psum.tile([128, 2 * BAND], fp32, tag="ps_k", bufs=2,
                               name=f"ps_kt{h}")
                     for h in range(2)]
            kt_sb = work.tile([128, KC, BAND], fp16)

            def kt_mm(m, k, start, stop):
                nc.tensor.matmul(
                    ps_kt[m // 2][:, (m % 2) * BAND:(m % 2 + 1) * BAND],
                    lhsT=wkt[:, k, m * 128:(m + 1) * 128],
                    rhs=xt(k),
                    start=start, stop=stop, skip_group_check=True,
                )

            def kt_drain(m):
                src = ps_kt[m // 2][:, (m % 2) * BAND:(m % 2 + 1) * BAND]
                if m % 2 == 0:
                    nc.scalar.copy(out=kt_sb[:, m, :], in_=src)
                else:
                    nc.vector.tensor_copy(out=kt_sb[:, m, :], in_=src)

            for m in (0, 2):
                for k in range(KC):
                    kt_mm(m, k, start=(k == 0), stop=(k == KC - 1))
                kt_drain(m)
            for m in (1, 3):
                for k in range(KC):
                    kt_mm(m, k, start=(k == 0), stop=(k == KC - 1))
                kt_drain(m)
            dummies(4)

            # ---- QT[m][c_out 128, r 128] = Wq x_q^T, same bank pattern
            # (bankC m0 then m1, bankD m2 then m3, k-halves pipelined).
            # st/Z/U later rotate through these two banks.
            ps_qt = [psum.tile([128, 2 * RCH], fp32, tag="ps_q", bufs=2,
                               name=f"ps_qt{h}")
                     for h in range(2)]
            qt_sb = work.tile([128, KC, RCH], fp16)

            def qt_mm(m, k, start, stop):
                nc.tensor.matmul(
                    ps_qt[m // 2][:, (m % 2) * RCH:(m % 2 + 1) * RCH],
                    lhsT=wqt[:, k, m * 128:(m + 1) * 128],
                    rhs=xt(k)[:, 64:64 + RCH],
                    start=start, stop=stop, skip_group_check=True,
                )

            def qt_drain(m):
                src = ps_qt[m // 2][:, (m % 2) * RCH:(m % 2 + 1) * RCH]
                if m % 2 == 0:
                    nc.scalar.copy(out=qt_sb[:, m, :], in_=src)
                else:
                    nc.vector.tensor_copy(out=qt_sb[:, m, :], in_=src)

            for m in (0, 2):
                for k in range(KC):
                    qt_mm(m, k, start=(k == 0), stop=(k == KC - 1))
                qt_drain(m)
            for m in (1, 3):
                for k in range(KC):
                    qt_mm(m, k, start=(k == 0), stop=(k == KC - 1))
                qt_drain(m)

            # ---- st[jc][j 128, r 128] = K Q^T (scores TRANSPOSED), then
            # E^T = exp(st + kb[j] - 40) straight to SBUF bf16.
            et_sb = work.tile([128, JC, RCH], bf16)
            ps_st = psum.tile([128, JC * RCH], fp32, tag="ps_q", bufs=2)
            for jc in range(JC):
                for m in range(KC):
                    nc.tensor.matmul(
                        ps_st[:, jc * RCH:(jc + 1) * RCH],
                        lhsT=kt_sb[:, m, jc * 128:(jc + 1) * 128],
                        rhs=qt_sb[:, m, :],
                        start=(m == 0),
                        stop=(m == KC - 1),
                        skip_group_check=True,
                    )
                nc.scalar.activation(
                    out=et_sb[:, jc, :], in_=ps_st[:, jc * RCH:(jc + 1) * RCH],
                    func=AFT.Exp, bias=kb[:, jc:jc + 1], scale=1.0,
                )

            # ---- V[jc][j 128, c 512] = x_band Wv^T, k-order 2,3,0,1
            # matching wv DMA arrival; k2,k3 fill the exp round-trip.
            ps_v = [psum.tile([128, C], fp32, tag="ps_v", bufs=2,
                              name=f"ps_v{jc}")
                    for jc in range(JC)]
            for k in (2, 3):
                for jc in range(JC):
                    nc.tensor.matmul(
                        ps_v[jc],
                        lhsT=xt(k)[:, jc * 128:(jc + 1) * 128],
                        rhs=wvt(k),
                        start=(k == 2),
                        stop=False,
                    )

            # ---- Z[tch][t 128, r 128] = Cm^T E^T;  W = mw / Z
            w_sb = work.tile([128, JC, RCH], bf16)
            rz = work.tile([128, JC, RCH], fp32)
            ps_z = psum.tile([128, JC * RCH], fp32, tag="ps_q", bufs=2)
            for tch in range(JC):
                for jc in range(JC):
                    nc.tensor.matmul(
                        ps_z[:, tch * RCH:(tch + 1) * RCH],
                        lhsT=cm[:, jc, tch * 128:(tch + 1) * 128],
                        rhs=et_sb[:, jc, :],
                        start=(jc == 0),
                        stop=(jc == JC - 1),
                        skip_group_check=True,
                    )
                nc.vector.reciprocal_approx_fast(
                    out=rz[:, tch, :], in_=ps_z[:, tch * RCH:(tch + 1) * RCH])
                nc.vector.tensor_mul(
                    w_sb[:, tch, :], rz[:, tch, :], mw[:, tch, :]
                )

            # ---- V k0,k1 (fills the W round-trip) + drains
            for k in (0, 1):
                for jc in range(JC):
                    nc.tensor.matmul(
                        ps_v[jc],
                        lhsT=xt(k)[:, jc * 128:(jc + 1) * 128],
                        rhs=wvt(k),
                        start=False,
                        stop=(k == 1),
                    )
            v_sb = work.tile([128, JC, C], fp16)
            for jc in range(JC):
                nc.vector.tensor_copy(
                    out=v_sb[:, jc, 0:256], in_=ps_v[jc][:, 0:256])
                nc.scalar.copy(
                    out=v_sb[:, jc, 256:512], in_=ps_v[jc][:, 256:512])

            # ---- U[jc][j 128, r 128] = Cm W;  A = E^T * U
            a_sb = work.tile([128, JC, RCH], fp16)
            ps_u = psum.tile([128, JC * RCH], fp32, tag="ps_q", bufs=2)
            for jc in range(JC):
                for tch in range(JC):
                    nc.tensor.matmul(
                        ps_u[:, jc * RCH:(jc + 1) * RCH],
                        lhsT=cmt[:, tch, jc * 128:(jc + 1) * 128],
                        rhs=w_sb[:, tch, :],
                        start=(tch == 0),
                        stop=(tch == JC - 1),
                        skip_group_check=True,
                    )
                nc.vector.tensor_mul(
                    a_sb[:, jc, :], ps_u[:, jc * RCH:(jc + 1) * RCH],
                    et_sb[:, jc, :]
                )

            # ---- out[r 128, c 512] = A^T V (bv adds on the host), fp16,
            # in 2 column halves so the first DMA overlaps the second.
            o_sb = work.tile([128, C], fp16)
            for ch in range(2):
                cs = ch * (C // 2)
                ps_o = psum.tile([128, C // 2], fp32, tag="ps_o", bufs=2,
                                 name=f"ps_o{ch}")
                for jc in range(JC):
                    nc.tensor.matmul(
                        ps_o,
                        lhsT=a_sb[:, jc, :],
                        rhs=v_sb[:, jc, cs:cs + C // 2],
                        start=(jc == 0),
                        stop=(jc == JC - 1),
                    )
                if ch == 0:
                    nc.vector.tensor_copy(
                        out=o_sb[:, cs:cs + C // 2], in_=ps_o)
                    nc.sync.dma_start(
                        out=out_d[:, cs:cs + C // 2],
                        in_=o_sb[:, cs:cs + C // 2])
                else:
                    nc.scalar.copy(out=o_sb[:, cs:cs + C // 2], in_=ps_o)
                    nc.scalar.dma_start(
                        out=out_d[:, cs:cs + C // 2],
                        in_=o_sb[:, cs:cs + C // 2])


def _pack128(arr):
    """[n*128, f] row-chunked -> [128, n*f] (chunk-major along free axis)."""
    n = arr.shape[0] // 128
    return np.ascontiguousarray(
        arr.reshape(n, 128, -1).transpose(1, 0, 2).reshape(128, -1)
    )


def _host_prep(image_features, Wq, bq, Wk, bk, Wv, bv, sample_idx):
    """Build the 8 per-core input blobs (pure index/layout work plus one
    tiny matvec kb = x_band @ (Wk^T bq) that folds bq into the exp bias)."""
    x = np.asarray(image_features, np.float32)
    sample_idx = np.asarray(sample_idx)

    # per-tile multiplicities -> banded count matrix Cm[j, t] = m_t[j - t]
    mod = (sample_idx % W).astype(np.int64)                  # [T, S]
    m = np.zeros((T, W), np.float32)
    np.add.at(m, (np.arange(T)[:, None], mod), 1.0)
    m += 1.0
    Cm = np.zeros((N, N), np.float32)
    rows = np.arange(T)
    for w in range(W):
        Cm[rows + w, rows] = m[:, w]

    pos = np.arange(N)
    counts = (np.minimum(pos, N - W) - np.maximum(pos - W + 1, 0) + 1)

    # padded versions for uniform band slicing
    XTp = np.zeros((B, C, N + 2 * 64), np.float16)
    for b in range(B):
        XTp[b, :, 64:64 + N] = x[b].T.astype(np.float16)
    Cmp = np.zeros((N + 2 * 64, N + 2 * 64), np.float32)
    Cmp[64:64 + N, 64:64 + N] = Cm

    wqt_p = _pack128(np.asarray(Wq, np.float32).T.astype(np.float16))
    wkt_p = _pack128(np.asarray(Wk, np.float32).T.astype(np.float16))
    wvt_p = _pack128(np.asarray(Wv, np.float32).T.astype(np.float16))
    # g = Wk^T bq; kb_full[padded j] = fp16(x_pad)[j] . g
    g = (np.asarray(Wk, np.float32).T @ np.asarray(bq, np.float32))
    kb_full = np.zeros((B, N + 2 * 64), np.float32)
    for b in range(B):
        kb_full[b] = XTp[b].astype(np.float32).T @ g

    in_maps = []
    for core in range(NCORES):
        b, rc = divmod(core, NCORES // B)
        r0 = rc * RCH
        xt = XTp[b, :, r0:r0 + BAND]
        cmb = np.ascontiguousarray(Cmp[r0:r0 + BAND, r0:r0 + BAND])
        # all-zero columns (padded t) would give Z=0 -> 1/0*mask = NaN on
        # device; a diagonal 1 keeps Z finite there and is masked out of W
        zero_cols = ~cmb.any(axis=0)
        cmb[zero_cols, zero_cols] = 1.0
        tl = np.arange(BAND)
        rl = np.arange(RCH)
        tg = r0 - 64 + tl
        rg = r0 + rl
        d = rg[None, :] - tg[:, None]
        valid = (d >= 0) & (d <= W - 1) & (tg[:, None] >= 0) & (tg[:, None] <= T - 1)
        maskw = np.where(
            valid, 1.0 / counts[rg][None, :], 0.0
        ).astype(np.float16)
        kbias = (kb_full[b, r0:r0 + BAND] + ESHIFT).astype(np.float32)

        blob = np.zeros((128, F16), np.float16)
        blob[:, OFF_XT:OFF_XT + KC * BAND] = _pack128(xt)
        blob[:, OFF_WQT:OFF_WQT + KC * C] = wqt_p
        blob[:, OFF_WKT:OFF_WKT + KC * C] = wkt_p
        blob[:, OFF_WVT01:OFF_WVT01 + 2 * C] = wvt_p[:, :2 * C]
        blob[:, OFF_WVT23:OFF_WVT23 + 2 * C] = wvt_p[:, 2 * C:]
        blob[:, OFF_MW:OFF_MW + JC * RCH] = _pack128(maskw)
        blobv = blob.view(np.uint16)
        blobv[:, OFF_KB:OFF_KB + 4] = (
            kbias.reshape(JC, 128).T.copy().view(np.uint16))

        cmu8 = np.zeros((128, NCM), np.uint8)
        cmu8[:, 0:JC * BAND] = _pack128(cmb.astype(np.uint8))
        cmu8[:, JC * BAND:] = _pack128(
            np.ascontiguousarray(cmb.T).astype(np.uint8))
        in_maps.append({"blob": blob, "cmu8": cmu8})
    return in_maps


def run_on_cores(in_maps, trace=False, trace_cores=None):
    from concourse.bass_utils import run_bass_kernel_spmd

    if "nc" not in _CACHE:
        _CACHE["nc"] = _build_program()
    nc = _CACHE["nc"]
    return run_bass_kernel_spmd(
        nc, in_maps, list(range(NCORES)), trace=trace,
        trace_cores=(trace_cores or [0]) if trace else None,
    )


def kernel(image_features, Wq, bq, Wk, bk, Wv, bv, sample_idx):
    in_maps = _host_prep(image_features, Wq, bq, Wk, bk, Wv, bv, sample_idx)
    res = run_on_cores(in_maps, trace=False)
    bv32 = np.asarray(bv, np.float32)[None, :]
    out = np.empty((B, N, C), np.float32)
    for core in range(NCORES):
        b, rc = divmod(core, NCORES // B)
        out[b, rc * RCH:(rc + 1) * RCH, :] = (
            res.results[core]["out"].astype(np.float32) + bv32)
    return out


# revision 21
# speedup vs baseline: 1.3875x; 1.0707x over previous
"""Trainium2 Bass kernel for ConsistentSelfAttentionTile.

Reference semantics: T=449 overlapping 64-token tiles; each tile attends to
352 KV tokens = 288 sampled (from a 9x replication of the tile) + the tile
itself; outputs overlap-add, then divide by overlap counts.

Algebraic collapse (same as the verified baseline):
  * rep[:, idx, :] == tile[:, idx % 64, :], so sampled KV tokens are tile
    rows with multiplicities m_t[w] = 1 + #{s : idx[t,s] % 64 == w}.
  * All per-tile 64x64 score blocks are diagonal blocks of one banded
    512x512 score matrix S = Q K^T (band |i-j| <= 63).
  * With E^T = exp(S^T + kb - 40), Cm[j,t] = m_t[j-t] (banded), the
    tile-softmax + overlap-add + count-divide collapses to
        Z = Cm^T E^T; W = mask/(counts * Z); U = Cm W; out = (E^T*U)^T V
    The constant -40 shift (vs a per-row max) cancels exactly; kb[j] =
    K[j].bq carries the Q bias (host-precomputed as x_band @ (Wk^T bq),
    exact), so Q/K projections are bias-free on device.
  * bk drops exactly (softmax-invariant); bv adds on the HOST to the
    returned output (attention rows sum to exactly 1), so V is bias-free.

v4 design notes (baseline 37.9us -> v3 31.3us -> this):
  * Input DMA: 9 unchained FIFO pieces on the two HWDGE rings (sync +
    scalar), ordered by need-time; x/wq/wk ship in k-chunk halves so the
    projections start accumulating as soon as each half lands. Cm/CmT
    ship as uint8 and cast to bf16 by a GPSIMD (SWDGE) DMA on its own
    ring. Output is fp16 (host casts back) in 2 halves on 2 rings.
  * PE runs at 1.2 GHz until the HAM sees ~3.4us of CONTINUOUS activity
    (idle gaps re-throttle): dependency-free warm-up matmuls run under
    the DMA lead-in and between arrival-gated phases.
  * PSUM: one OPEN accumulation group per bank at a time (start=True
    clobbers sibling has_written). 8 banks: KT 2, QT->st->Z->U rotate 2,
    V 2, out 2 (shared with warm-up).
  * exp bias rides per-partition (kb - 40); 1/Z via the ~5x-faster
    reciprocal_approx_fast; drains split across ACT and DVE.

Sharding: 8 cores = 2 batches x 4 row-chunks of 128 output rows, each core
fully independent on a 256-column band of its batch's sequence.
"""

import os
import sys

import numpy as np

try:
    import ml_dtypes
except ImportError:
    ml_dtypes = None

for _p in ("/opt/trn_rl_repo",):
    if _p not in sys.path and os.path.isdir(_p):
        sys.path.insert(0, _p)

B, N, C, W = 2, 512, 512, 64
T = N - W + 1          # 449 tiles
RCH = 128              # output rows per core
NCORES = 8
BAND = 256             # per-core j/t band width (columns [r0-64, r0+192))
KC = C // 128          # 4 contraction chunks
JC = BAND // 128       # 2 band chunks
ESHIFT = -40.0         # constant exp shift (cancels exactly; keeps the
                       # activation-table inputs in the proven-negative range)

# blob layout (fp16 columns; kb is a [128,2] fp32 bitcast view). Order is
# the DMA piece order: sync ring x+kb | G halves, scalar ring wv halves;
# cm/cmt (uint8) and mw ride the GPSIMD SWDGE ring. G = Wq^T Wk folds
# both score-path weight matrices into one (S = x_q G x_band^T), so no K
# projection exists on device at all.
OFF_XT = 0                       # [128, 4, 256] x^T band
OFF_KB = OFF_XT + KC * BAND      # [128, 2] fp32 exp bias kb[j]+ESHIFT
OFF_G = OFF_KB + 4               # [128, 4, 512] G = Wq^T Wk (d-chunk-major)
OFF_WVT = OFF_G + KC * C         # [128, 4, 512]
OFF_MW = OFF_WVT + KC * C        # [128, 2, 128] fp16 mask/counts
F16 = OFF_MW + JC * RCH

NCM = 2 * JC * BAND              # cm | cmt as uint8, cast to bf16 on-chip

_CACHE = {}


def _slim_drain_and_barrier(self, tick_clock, wait_clock):
    """Cheaper TileContext exit: final drain covers only DMA-queue
    completion, then one sem-only barrier + semaphore reset."""
    from concourse.vector_clock import ScopedClock, VectorClock
    from concourse.tile_scheduler import dmasw_start_idx, N_PROCS

    g = tick_clock.global_clock
    dma_clock = VectorClock()
    for idx in range(dmasw_start_idx, N_PROCS):
        t = g.peek_next(idx) - 1
        if t > 0:
            dma_clock.require_at_least(idx, t)
    drain_inst = self.nc.sync.drain()
    wait_clock.add_sem_waits(drain_inst.ins, ScopedClock({None: dma_clock}))
    self.nc.all_engine_barrier(sem_only=True)
    popped = self.nc._tile_sem_poison_stack.pop()
    assert popped is self._sem_poison
    self.nc.clear_and_free_semaphores(list(self.sems.allocated().values()))


def _build_program():
    import concourse.bacc as bacc
    import concourse.mybir as mybir
    import concourse.tile as tile

    fp16 = mybir.dt.float16
    uint8 = mybir.dt.uint8
    # Skip Bass's preamble all-engine barrier (Tile sems carry all real
    # deps), so the input DMA issues ~5us earlier.
    orig_aeb = bacc.Bacc.all_engine_barrier

    def _noop_aeb(self, *, sem_only=False):
        return None

    bacc.Bacc.all_engine_barrier = _noop_aeb
    try:
        nc = bacc.Bacc("TRN2", target_bir_lowering=False, debug=False)
    finally:
        bacc.Bacc.all_engine_barrier = orig_aeb

    blob_d = nc.declare_dram_parameter("blob", [128, F16], fp16, isOutput=False)
    cm_d = nc.declare_dram_parameter("cmu8", [128, NCM], uint8, isOutput=False)
    out_d = nc.declare_dram_parameter("out", [RCH, C], fp16, isOutput=True)

    orig_dab = tile.TileContext._drain_and_barrier
    tile.TileContext._drain_and_barrier = _slim_drain_and_barrier
    try:
        _emit_body(nc, tile, mybir, blob_d, cm_d, out_d)
    finally:
        tile.TileContext._drain_and_barrier = orig_dab

    nc.compile()
    return nc


def _emit_body(nc, tile, mybir, blob_d, cm_d, out_d):
    fp32 = mybir.dt.float32
    fp16 = mybir.dt.float16
    bf16 = mybir.dt.bfloat16
    AFT = mybir.ActivationFunctionType

    with tile.TileContext(nc) as tc:
        with (
            tc.tile_pool(name="consts", bufs=1) as consts,
            tc.tile_pool(name="work", bufs=1) as work,
            tc.tile_pool(name="psum", bufs=1, space="PSUM") as psum,
        ):
            blob = consts.tile([128, F16], fp16)
            cmu_sb = consts.tile([128, NCM], mybir.dt.uint8)
            cmx_sb = consts.tile([128, NCM], bf16)
            dum_src = work.tile([128, 256], fp16)
            nc.gpsimd.memset(dum_src, 0.0)

            # ---- input DMA: unchained FIFO pieces; each ring drains in
            # instruction order and the SDMA engines round-robin rings.
            sync_pieces = [
                (OFF_XT, OFF_G),                 # x + kb
                (OFF_G, OFF_G + 2 * C),          # G k0,k1
                (OFF_G + 2 * C, OFF_WVT),        # G k2,k3
            ]
            scalar_pieces = [
                (OFF_WVT, OFF_WVT + 2 * C),      # wv k0,k1
                (OFF_WVT + 2 * C, OFF_MW),       # wv k2,k3
            ]
            for lo, hi in sync_pieces:
                nc.sync.dma_start(out=blob[:, lo:hi], in_=blob_d[:, lo:hi])
            for lo, hi in scalar_pieces:
                nc.scalar.dma_start(out=blob[:, lo:hi], in_=blob_d[:, lo:hi])
            # cm/cmt (uint8, cast to bf16 on-chip by the otherwise-idle
            # GPSIMD) and mw ride the SWDGE ring
            nc.gpsimd.dma_start(out=cmu_sb, in_=cm_d[:])
            nc.gpsimd.dma_start(
                out=blob[:, OFF_MW:F16], in_=blob_d[:, OFF_MW:F16])
            nc.gpsimd.tensor_copy(out=cmx_sb, in_=cmu_sb)

            xt_v = blob[:, OFF_XT:OFF_XT + KC * BAND].rearrange(
                "p (k j) -> p k j", k=KC)

            def xt(k):
                return xt_v[:, k, :]

            g_sb = blob[:, OFF_G:OFF_G + KC * C].rearrange(
                "p (k j) -> p k j", k=KC)
            wvt_v = blob[:, OFF_WVT:OFF_WVT + KC * C].rearrange(
                "p (k j) -> p k j", k=KC)

            def wvt(k):
                return wvt_v[:, k, :]
            kb = blob[:, OFF_KB:OFF_KB + 4].bitcast(fp32)
            mw = blob[:, OFF_MW:OFF_MW + JC * RCH].rearrange(
                "p (k r) -> p k r", k=JC)
            cm = cmx_sb[:, 0:JC * BAND].rearrange("p (k t) -> p k t", k=JC)
            cmt = cmx_sb[:, JC * BAND:NCM].rearrange("p (k j) -> p k j", k=JC)

            # ---- PE warm-up: dependency-free matmuls keep the HAM clock
            # gate fed while DMA streams (PE is 1.2 GHz until ~3.4us of
            # continuous activity; gaps re-throttle).
            ps_dum = psum.tile([128, 256], fp32, tag="ps_o", bufs=2)

            def dummies(n):
                for _ in range(n):
                    nc.tensor.matmul(
                        ps_dum, lhsT=dum_src[:, 0:128], rhs=dum_src,
                        start=True, stop=True,
                    )

            dummies(10)

            # ---- V[jc][j 128, c 512] = x_band Wv^T, k-order matching
            # the wv DMA halves (PE's first real work; doubles as warm-up)
            ps_v = [psum.tile([128, C], fp32, tag="ps_v", bufs=2,
                              name=f"ps_v{jc}")
                    for jc in range(JC)]
            for k in range(KC):
                for jc in range(JC):
                    nc.tensor.matmul(
                        ps_v[jc],
                        lhsT=xt(k)[:, jc * 128:(jc + 1) * 128],
                        rhs=wvt(k),
                        start=(k == 0),
                        stop=(k == KC - 1),
                    )
            dummies(2)

            # ---- HH[m][e 128, r 128] = G^T x_q^T (the whole score path's
            # only projection). 4 banks -> 4 independent groups, k-major
            # so each G half releases 8 matmuls.
            ps_hh = [psum.tile([128, RCH], fp32, tag="ps_q", bufs=4,
                               name=f"ps_hh{m}")
                     for m in range(KC)]
            hh_sb = work.tile([128, KC, RCH], fp16)
            for k in range(KC):
                for m in range(KC):
                    nc.tensor.matmul(
                        ps_hh[m],
                        lhsT=g_sb[:, k, m * 128:(m + 1) * 128],
                        rhs=xt(k)[:, 64:64 + RCH],
                        start=(k == 0),
                        stop=(k == KC - 1),
                    )
            for m in range(KC):
                if m % 2 == 0:
                    nc.scalar.copy(out=hh_sb[:, m, :], in_=ps_hh[m])
                else:
                    nc.vector.tensor_copy(out=hh_sb[:, m, :], in_=ps_hh[m])

            # ---- V drains (out needs them later than st needs hh)
            v_sb = work.tile([128, JC, C], fp16)
            for jc in range(JC):
                nc.vector.tensor_copy(
                    out=v_sb[:, jc, 0:256], in_=ps_v[jc][:, 0:256])
                nc.scalar.copy(
                    out=v_sb[:, jc, 256:512], in_=ps_v[jc][:, 256:512])

            # ---- st[jc][j 128, r 128] = K Q^T (scores TRANSPOSED), then
            # E^T = exp(st + kb[j] - 40) straight to SBUF bf16.
            et_sb = work.tile([128, JC, RCH], bf16)
            ps_st = psum.tile([128, JC * RCH], fp32, tag="ps_q", bufs=4)
            for jc in range(JC):
                for m in range(KC):
                    nc.tensor.matmul(
                        ps_st[:, jc * RCH:(jc + 1) * RCH],
                        lhsT=xt(m)[:, jc * 128:(jc + 1) * 128],
                        rhs=hh_sb[:, m, :],
                        start=(m == 0),
                        stop=(m == KC - 1),
                        skip_group_check=True,
                    )
                nc.scalar.activation(
                    out=et_sb[:, jc, :], in_=ps_st[:, jc * RCH:(jc + 1) * RCH],
                    func=AFT.Exp, bias=kb[:, jc:jc + 1], scale=1.0,
                )

            # ---- Z[tch][t 128, r 128] = Cm^T E^T;  W = mw / Z
            w_sb = work.tile([128, JC, RCH], bf16)
            rz = work.tile([128, JC, RCH], fp32)
            ps_z = psum.tile([128, JC * RCH], fp32, tag="ps_q", bufs=4)
            for tch in range(JC):
                for jc in range(JC):
                    nc.tensor.matmul(
                        ps_z[:, tch * RCH:(tch + 1) * RCH],
                        lhsT=cm[:, jc, tch * 128:(tch + 1) * 128],
                        rhs=et_sb[:, jc, :],
                        start=(jc == 0),
                        stop=(jc == JC - 1),
                        skip_group_check=True,
                    )
                nc.vector.reciprocal_approx_fast(
                    out=rz[:, tch, :], in_=ps_z[:, tch * RCH:(tch + 1) * RCH])
                nc.vector.tensor_mul(
                    w_sb[:, tch, :], rz[:, tch, :], mw[:, tch, :]
                )

            # ---- U[jc][j 128, r 128] = Cm W;  A = E^T * U
            a_sb = work.tile([128, JC, RCH], fp16)
            ps_u = psum.tile([128, JC * RCH], fp32, tag="ps_q", bufs=4)
            for jc in range(JC):
                for tch in range(JC):
                    nc.tensor.matmul(
                        ps_u[:, jc * RCH:(jc + 1) * RCH],
                        lhsT=cmt[:, tch, jc * 128:(jc + 1) * 128],
                        rhs=w_sb[:, tch, :],
                        start=(tch == 0),
                        stop=(tch == JC - 1),
                        skip_group_check=True,
                    )
                nc.vector.tensor_mul(
                    a_sb[:, jc, :], ps_u[:, jc * RCH:(jc + 1) * RCH],
                    et_sb[:, jc, :]
                )

            # ---- out[r 128, c 512] = A^T V (bv adds on the host), fp16,
            # in 2 column halves so the first DMA overlaps the second.
            o_sb = work.tile([128, C], fp16)
            for ch in range(2):
                cs = ch * (C // 2)
                ps_o = psum.tile([128, C // 2], fp32, tag="ps_o", bufs=2,
                                 name=f"ps_o{ch}")
                for jc in range(JC):
                    nc.tensor.matmul(
                        ps_o,
                        lhsT=a_sb[:, jc, :],
                        rhs=v_sb[:, jc, cs:cs + C // 2],
                        start=(jc == 0),
                        stop=(jc == JC - 1),
                    )
                if ch == 0:
                    nc.vector.tensor_copy(
                        out=o_sb[:, cs:cs + C // 2], in_=ps_o)
                    nc.sync.dma_start(
                        out=out_d[:, cs:cs + C // 2],
                        in_=o_sb[:, cs:cs + C // 2])
                else:
                    nc.scalar.copy(out=o_sb[:, cs:cs + C // 2], in_=ps_o)
                    nc.scalar.dma_start(
                        out=out_d[:, cs:cs + C // 2],
                        in_=o_sb[:, cs:cs + C // 2])


def _pack128(arr):
    """[n*128, f] row-chunked -> [128, n*f] (chunk-major along free axis)."""
    n = arr.shape[0] // 128
    return np.ascontiguousarray(
        arr.reshape(n, 128, -1).transpose(1, 0, 2).reshape(128, -1)
    )


def _host_prep(image_features, Wq, bq, Wk, bk, Wv, bv, sample_idx):
    """Build the 8 per-core input blobs (pure index/layout work plus one
    tiny matvec kb = x_band @ (Wk^T bq) that folds bq into the exp bias)."""
    x = np.asarray(image_features, np.float32)
    sample_idx = np.asarray(sample_idx)

    # per-tile multiplicities -> banded count matrix Cm[j, t] = m_t[j - t]
    mod = (sample_idx % W).astype(np.int64)                  # [T, S]
    m = np.zeros((T, W), np.float32)
    np.add.at(m, (np.arange(T)[:, None], mod), 1.0)
    m += 1.0
    Cm = np.zeros((N, N), np.float32)
    rows = np.arange(T)
    for w in range(W):
        Cm[rows + w, rows] = m[:, w]

    pos = np.arange(N)
    counts = (np.minimum(pos, N - W) - np.maximum(pos - W + 1, 0) + 1)

    # padded versions for uniform band slicing
    XTp = np.zeros((B, C, N + 2 * 64), np.float16)
    for b in range(B):
        XTp[b, :, 64:64 + N] = x[b].T.astype(np.float16)
    Cmp = np.zeros((N + 2 * 64, N + 2 * 64), np.float32)
    Cmp[64:64 + N, 64:64 + N] = Cm

    Wq32 = np.asarray(Wq, np.float32)
    Wk32 = np.asarray(Wk, np.float32)
    G = Wq32.T @ Wk32                       # S = x_q G x_band^T
    g_p = _pack128(G.astype(np.float16))
    wvt_p = _pack128(np.asarray(Wv, np.float32).T.astype(np.float16))
    # g = Wk^T bq; kb_full[padded j] = fp16(x_pad)[j] . g
    g = (np.asarray(Wk, np.float32).T @ np.asarray(bq, np.float32))
    kb_full = np.zeros((B, N + 2 * 64), np.float32)
    for b in range(B):
        kb_full[b] = XTp[b].astype(np.float32).T @ g

    in_maps = []
    for core in range(NCORES):
        b, rc = divmod(core, NCORES // B)
        r0 = rc * RCH
        xt = XTp[b, :, r0:r0 + BAND]
        cmb = np.ascontiguousarray(Cmp[r0:r0 + BAND, r0:r0 + BAND])
        # all-zero columns (padded t) would give Z=0 -> 1/0*mask = NaN on
        # device; a diagonal 1 keeps Z finite there and is masked out of W
        zero_cols = ~cmb.any(axis=0)
        cmb[zero_cols, zero_cols] = 1.0
        tl = np.arange(BAND)
        rl = np.arange(RCH)
        tg = r0 - 64 + tl
        rg = r0 + rl
        d = rg[None, :] - tg[:, None]
        valid = (d >= 0) & (d <= W - 1) & (tg[:, None] >= 0) & (tg[:, None] <= T - 1)
        maskw = np.where(
            valid, 1.0 / counts[rg][None, :], 0.0
        ).astype(np.float16)
        kbias = (kb_full[b, r0:r0 + BAND] + ESHIFT).astype(np.float32)

        blob = np.zeros((128, F16), np.float16)
        blob[:, OFF_XT:OFF_XT + KC * BAND] = _pack128(xt)
        blob[:, OFF_G:OFF_G + KC * C] = g_p
        blob[:, OFF_WVT:OFF_WVT + KC * C] = wvt_p
        blob[:, OFF_MW:OFF_MW + JC * RCH] = _pack128(maskw)
        blobv = blob.view(np.uint16)
        blobv[:, OFF_KB:OFF_KB + 4] = (
            kbias.reshape(JC, 128).T.copy().view(np.uint16))

        cmu8 = np.zeros((128, NCM), np.uint8)
        cmu8[:, 0:JC * BAND] = _pack128(cmb.astype(np.uint8))
        cmu8[:, JC * BAND:] = _pack128(
            np.ascontiguousarray(cmb.T).astype(np.uint8))
        in_maps.append({"blob": blob, "cmu8": cmu8})
    return in_maps


def run_on_cores(in_maps, trace=False, trace_cores=None):
    from concourse.bass_utils import run_bass_kernel_spmd

    if "nc" not in _CACHE:
        _CACHE["nc"] = _build_program()
    nc = _CACHE["nc"]
    return run_bass_kernel_spmd(
        nc, in_maps, list(range(NCORES)), trace=trace,
        trace_cores=(trace_cores or [0]) if trace else None,
    )


def kernel(image_features, Wq, bq, Wk, bk, Wv, bv, sample_idx):
    in_maps = _host_prep(image_features, Wq, bq, Wk, bk, Wv, bv, sample_idx)
    res = run_on_cores(in_maps, trace=False)
    bv32 = np.asarray(bv, np.float32)[None, :]
    out = np.empty((B, N, C), np.float32)
    for core in range(NCORES):
        b, rc = divmod(core, NCORES // B)
        out[b, rc * RCH:(rc + 1) * RCH, :] = (
            res.results[core]["out"].astype(np.float32) + bv32)
    return out


# revision 22
# speedup vs baseline: 1.5379x; 1.1084x over previous
"""Trainium2 Bass kernel for ConsistentSelfAttentionTile.

Reference semantics: T=449 overlapping 64-token tiles; each tile attends to
352 KV tokens = 288 sampled (from a 9x replication of the tile) + the tile
itself; outputs overlap-add, then divide by overlap counts.

Algebraic collapse (same as the verified baseline):
  * rep[:, idx, :] == tile[:, idx % 64, :], so sampled KV tokens are tile
    rows with multiplicities m_t[w] = 1 + #{s : idx[t,s] % 64 == w}.
  * All per-tile 64x64 score blocks are diagonal blocks of one banded
    512x512 score matrix S = Q K^T (band |i-j| <= 63).
  * With E^T = exp(S^T + kb - 40), Cm[j,t] = m_t[j-t] (banded), the
    tile-softmax + overlap-add + count-divide collapses to
        Z = Cm^T E^T; W = mask/(counts * Z); U = Cm W; out = (E^T*U)^T V
    The constant -40 shift (vs a per-row max) cancels exactly; kb[j] =
    K[j].bq carries the Q bias (host-precomputed as x_band @ (Wk^T bq),
    exact), so Q/K projections are bias-free on device.
  * bk drops exactly (softmax-invariant); bv adds on the HOST to the
    returned output (attention rows sum to exactly 1), so V is bias-free.

v4 design notes (baseline 37.9us -> v3 31.3us -> this):
  * Input DMA: 9 unchained FIFO pieces on the two HWDGE rings (sync +
    scalar), ordered by need-time; x/wq/wk ship in k-chunk halves so the
    projections start accumulating as soon as each half lands. Cm/CmT
    ship as uint8 and cast to bf16 by a GPSIMD (SWDGE) DMA on its own
    ring. Output is fp16 (host casts back) in 2 halves on 2 rings.
  * PE runs at 1.2 GHz until the HAM sees ~3.4us of CONTINUOUS activity
    (idle gaps re-throttle): dependency-free warm-up matmuls run under
    the DMA lead-in and between arrival-gated phases.
  * PSUM: one OPEN accumulation group per bank at a time (start=True
    clobbers sibling has_written). 8 banks: KT 2, QT->st->Z->U rotate 2,
    V 2, out 2 (shared with warm-up).
  * exp bias rides per-partition (kb - 40); 1/Z via the ~5x-faster
    reciprocal_approx_fast; drains split across ACT and DVE.

Sharding: 8 cores = 2 batches x 4 row-chunks of 128 output rows, each core
fully independent on a 256-column band of its batch's sequence.
"""

import os
import sys

import numpy as np

try:
    import ml_dtypes
except ImportError:
    ml_dtypes = None

for _p in ("/opt/trn_rl_repo",):
    if _p not in sys.path and os.path.isdir(_p):
        sys.path.insert(0, _p)

B, N, C, W = 2, 512, 512, 64
T = N - W + 1          # 449 tiles
RCH = 128              # output rows per core
NCORES = 8
BAND = 256             # per-core j/t band width (columns [r0-64, r0+192))
KC = C // 128          # 4 contraction chunks
JC = BAND // 128       # 2 band chunks
ESHIFT = -40.0         # constant exp shift (cancels exactly; keeps the
                       # activation-table inputs in the proven-negative range)

# blob layout (fp16 columns; kb is a [128,2] fp32 bitcast view). Order is
# the DMA piece order: sync ring x+kb | G halves, scalar ring wv halves;
# cm/cmt (uint8) and mw ride the GPSIMD SWDGE ring. G = Wq^T Wk folds
# both score-path weight matrices into one (S = x_q G x_band^T), so no K
# projection exists on device at all.
OFF_XT = 0                       # [128, 4, 256] x^T band
OFF_KB = OFF_XT + KC * BAND      # [128, 2] fp32 exp bias kb[j]+ESHIFT
OFF_G = OFF_KB + 4               # [128, 4, 512] G = Wq^T Wk (d-chunk-major)
OFF_WVT = OFF_G + KC * C         # [128, 4, 512]
OFF_MW = OFF_WVT + KC * C        # [128, 2, 128] fp16 mask/counts
F16 = OFF_MW + JC * RCH

NCM = 2 * JC * BAND              # cm | cmt as uint8, cast to bf16 on-chip

_CACHE = {}


def _slim_drain_and_barrier(self, tick_clock, wait_clock):
    """Cheaper TileContext exit: final drain covers only DMA-queue
    completion, then one sem-only barrier + semaphore reset."""
    from concourse.vector_clock import ScopedClock, VectorClock
    from concourse.tile_scheduler import dmasw_start_idx, N_PROCS

    g = tick_clock.global_clock
    dma_clock = VectorClock()
    for idx in range(dmasw_start_idx, N_PROCS):
        t = g.peek_next(idx) - 1
        if t > 0:
            dma_clock.require_at_least(idx, t)
    drain_inst = self.nc.sync.drain()
    wait_clock.add_sem_waits(drain_inst.ins, ScopedClock({None: dma_clock}))
    self.nc.all_engine_barrier(sem_only=True)
    popped = self.nc._tile_sem_poison_stack.pop()
    assert popped is self._sem_poison
    self.nc.clear_and_free_semaphores(list(self.sems.allocated().values()))


def _build_program():
    import concourse.bacc as bacc
    import concourse.mybir as mybir
    import concourse.tile as tile

    fp16 = mybir.dt.float16
    uint8 = mybir.dt.uint8
    # Skip Bass's preamble all-engine barrier (Tile sems carry all real
    # deps), so the input DMA issues ~5us earlier.
    orig_aeb = bacc.Bacc.all_engine_barrier

    def _noop_aeb(self, *, sem_only=False):
        return None

    bacc.Bacc.all_engine_barrier = _noop_aeb
    try:
        nc = bacc.Bacc("TRN2", target_bir_lowering=False, debug=False)
    finally:
        bacc.Bacc.all_engine_barrier = orig_aeb

    blob_d = nc.declare_dram_parameter("blob", [128, F16], fp16, isOutput=False)
    cm_d = nc.declare_dram_parameter("cmu8", [128, NCM], uint8, isOutput=False)
    out_d = nc.declare_dram_parameter("out", [RCH, C], fp16, isOutput=True)

    orig_dab = tile.TileContext._drain_and_barrier
    tile.TileContext._drain_and_barrier = _slim_drain_and_barrier
    try:
        _emit_body(nc, tile, mybir, blob_d, cm_d, out_d)
    finally:
        tile.TileContext._drain_and_barrier = orig_dab

    nc.compile()
    return nc


def _emit_body(nc, tile, mybir, blob_d, cm_d, out_d):
    fp32 = mybir.dt.float32
    fp16 = mybir.dt.float16
    bf16 = mybir.dt.bfloat16
    AFT = mybir.ActivationFunctionType

    with tile.TileContext(nc) as tc:
        with (
            tc.tile_pool(name="consts", bufs=1) as consts,
            tc.tile_pool(name="work", bufs=1) as work,
            tc.tile_pool(name="psum", bufs=1, space="PSUM") as psum,
        ):
            blob = consts.tile([128, F16], fp16)
            cmu_sb = consts.tile([128, NCM], mybir.dt.uint8)
            cmx_sb = consts.tile([128, NCM], bf16)
            dum_src = work.tile([128, 256], fp16)
            nc.gpsimd.memset(dum_src, 0.0)

            # ---- input DMA: unchained FIFO pieces; each ring drains in
            # instruction order and the SDMA engines round-robin rings.
            sync_pieces = [
                (OFF_XT, OFF_G),                 # x + kb
                (OFF_G, OFF_G + 2 * C),          # G k0,k1
                (OFF_G + 2 * C, OFF_WVT),        # G k2,k3
            ]
            scalar_pieces = [
                (OFF_WVT, OFF_WVT + 2 * C),      # wv k0,k1
                (OFF_WVT + 2 * C, OFF_MW),       # wv k2,k3
            ]
            for lo, hi in sync_pieces:
                nc.sync.dma_start(out=blob[:, lo:hi], in_=blob_d[:, lo:hi])
            for lo, hi in scalar_pieces:
                nc.scalar.dma_start(out=blob[:, lo:hi], in_=blob_d[:, lo:hi])
            # cm/cmt (uint8, cast to bf16 on-chip by the otherwise-idle
            # GPSIMD) and mw ride the SWDGE ring
            nc.gpsimd.dma_start(out=cmu_sb, in_=cm_d[:])
            nc.gpsimd.dma_start(
                out=blob[:, OFF_MW:F16], in_=blob_d[:, OFF_MW:F16])
            nc.gpsimd.tensor_copy(out=cmx_sb, in_=cmu_sb)

            xt_v = blob[:, OFF_XT:OFF_XT + KC * BAND].rearrange(
                "p (k j) -> p k j", k=KC)

            def xt(k):
                return xt_v[:, k, :]

            g_sb = blob[:, OFF_G:OFF_G + KC * C].rearrange(
                "p (k j) -> p k j", k=KC)
            wvt_v = blob[:, OFF_WVT:OFF_WVT + KC * C].rearrange(
                "p (k j) -> p k j", k=KC)

            def wvt(k):
                return wvt_v[:, k, :]
            kb = blob[:, OFF_KB:OFF_KB + 4].bitcast(fp32)
            mw = blob[:, OFF_MW:OFF_MW + JC * RCH].rearrange(
                "p (k r) -> p k r", k=JC)
            cm = cmx_sb[:, 0:JC * BAND].rearrange("p (k t) -> p k t", k=JC)
            cmt = cmx_sb[:, JC * BAND:NCM].rearrange("p (k j) -> p k j", k=JC)

            # ---- PE warm-up: dependency-free matmuls keep the HAM clock
            # gate fed while DMA streams (PE is 1.2 GHz until ~3.4us of
            # continuous activity; gaps re-throttle).
            ps_dum = psum.tile([128, 256], fp32, tag="ps_o", bufs=2)

            def dummies(n):
                for _ in range(n):
                    nc.tensor.matmul(
                        ps_dum, lhsT=dum_src[:, 0:128], rhs=dum_src,
                        start=True, stop=True,
                    )

            dummies(22)

            # ---- V[jc][j 128, c 512] = x_band Wv^T, k-order matching
            # the wv DMA halves (PE's first real work; doubles as warm-up)
            ps_v = [psum.tile([128, C], fp32, tag="ps_v", bufs=2,
                              name=f"ps_v{jc}")
                    for jc in range(JC)]
            for k in range(KC):
                for jc in range(JC):
                    nc.tensor.matmul(
                        ps_v[jc],
                        lhsT=xt(k)[:, jc * 128:(jc + 1) * 128],
                        rhs=wvt(k),
                        start=(k == 0),
                        stop=(k == KC - 1),
                    )
            dummies(2)

            # ---- HH[m][e 128, r 128] = G^T x_q^T (the whole score path's
            # only projection). 4 banks -> 4 independent groups, k-major
            # so each G half releases 8 matmuls.
            ps_hh = [psum.tile([128, RCH], fp32, tag="ps_q", bufs=4,
                               name=f"ps_hh{m}")
                     for m in range(KC)]
            hh_sb = work.tile([128, KC, RCH], fp16)
            for k in range(KC):
                for m in range(KC):
                    nc.tensor.matmul(
                        ps_hh[m],
                        lhsT=g_sb[:, k, m * 128:(m + 1) * 128],
                        rhs=xt(k)[:, 64:64 + RCH],
                        start=(k == 0),
                        stop=(k == KC - 1),
                    )
            for m in range(KC):
                if m % 2 == 0:
                    nc.scalar.copy(out=hh_sb[:, m, :], in_=ps_hh[m])
                else:
                    nc.vector.tensor_copy(out=hh_sb[:, m, :], in_=ps_hh[m])

            # ---- V drains (out needs them later than st needs hh)
            v_sb = work.tile([128, JC, C], fp16)
            for jc in range(JC):
                nc.vector.tensor_copy(
                    out=v_sb[:, jc, 0:256], in_=ps_v[jc][:, 0:256])
                nc.scalar.copy(
                    out=v_sb[:, jc, 256:512], in_=ps_v[jc][:, 256:512])

            # ---- st[jc][j 128, r 128] = K Q^T (scores TRANSPOSED), then
            # E^T = exp(st + kb[j] - 40) straight to SBUF bf16.
            et_sb = work.tile([128, JC, RCH], bf16)
            ps_st = psum.tile([128, JC * RCH], fp32, tag="ps_q", bufs=4)
            for jc in range(JC):
                for m in range(KC):
                    nc.tensor.matmul(
                        ps_st[:, jc * RCH:(jc + 1) * RCH],
                        lhsT=xt(m)[:, jc * 128:(jc + 1) * 128],
                        rhs=hh_sb[:, m, :],
                        start=(m == 0),
                        stop=(m == KC - 1),
                        skip_group_check=True,
                    )
                nc.scalar.activation(
                    out=et_sb[:, jc, :], in_=ps_st[:, jc * RCH:(jc + 1) * RCH],
                    func=AFT.Exp, bias=kb[:, jc:jc + 1], scale=1.0,
                )
                dummies(2)

            # ---- Z[tch][t 128, r 128] = Cm^T E^T;  W = mw / Z
            w_sb = work.tile([128, JC, RCH], bf16)
            rz = work.tile([128, JC, RCH], fp32)
            ps_z = psum.tile([128, JC * RCH], fp32, tag="ps_q", bufs=4)
            for tch in range(JC):
                for jc in range(JC):
                    nc.tensor.matmul(
                        ps_z[:, tch * RCH:(tch + 1) * RCH],
                        lhsT=cm[:, jc, tch * 128:(tch + 1) * 128],
                        rhs=et_sb[:, jc, :],
                        start=(jc == 0),
                        stop=(jc == JC - 1),
                        skip_group_check=True,
                    )
                nc.vector.reciprocal_approx_fast(
                    out=rz[:, tch, :], in_=ps_z[:, tch * RCH:(tch + 1) * RCH])
                nc.vector.tensor_mul(
                    w_sb[:, tch, :], rz[:, tch, :], mw[:, tch, :]
                )
                dummies(2)

            # ---- U[jc][j 128, r 128] = Cm W;  A = E^T * U
            a_sb = work.tile([128, JC, RCH], fp16)
            ps_u = psum.tile([128, JC * RCH], fp32, tag="ps_q", bufs=4)
            for jc in range(JC):
                for tch in range(JC):
                    nc.tensor.matmul(
                        ps_u[:, jc * RCH:(jc + 1) * RCH],
                        lhsT=cmt[:, tch, jc * 128:(jc + 1) * 128],
                        rhs=w_sb[:, tch, :],
                        start=(tch == 0),
                        stop=(tch == JC - 1),
                        skip_group_check=True,
                    )
                nc.vector.tensor_mul(
                    a_sb[:, jc, :], ps_u[:, jc * RCH:(jc + 1) * RCH],
                    et_sb[:, jc, :]
                )
                dummies(2)

            # ---- out[r 128, c 512] = A^T V (bv adds on the host), fp16,
            # in 2 column halves so the first DMA overlaps the second.
            o_sb = work.tile([128, C], fp16)
            for ch in range(2):
                cs = ch * (C // 2)
                ps_o = psum.tile([128, C // 2], fp32, tag="ps_o", bufs=2,
                                 name=f"ps_o{ch}")
                for jc in range(JC):
                    nc.tensor.matmul(
                        ps_o,
                        lhsT=a_sb[:, jc, :],
                        rhs=v_sb[:, jc, cs:cs + C // 2],
                        start=(jc == 0),
                        stop=(jc == JC - 1),
                    )
                if ch == 0:
                    nc.vector.tensor_copy(
                        out=o_sb[:, cs:cs + C // 2], in_=ps_o)
                    nc.sync.dma_start(
                        out=out_d[:, cs:cs + C // 2],
                        in_=o_sb[:, cs:cs + C // 2])
                else:
                    nc.scalar.copy(out=o_sb[:, cs:cs + C // 2], in_=ps_o)
                    nc.scalar.dma_start(
                        out=out_d[:, cs:cs + C // 2],
                        in_=o_sb[:, cs:cs + C // 2])


def _pack128(arr):
    """[n*128, f] row-chunked -> [128, n*f] (chunk-major along free axis)."""
    n = arr.shape[0] // 128
    return np.ascontiguousarray(
        arr.reshape(n, 128, -1).transpose(1, 0, 2).reshape(128, -1)
    )


def _host_prep(image_features, Wq, bq, Wk, bk, Wv, bv, sample_idx):
    """Build the 8 per-core input blobs (pure index/layout work plus one
    tiny matvec kb = x_band @ (Wk^T bq) that folds bq into the exp bias)."""
    x = np.asarray(image_features, np.float32)
    sample_idx = np.asarray(sample_idx)

    # per-tile multiplicities -> banded count matrix Cm[j, t] = m_t[j - t]
    mod = (sample_idx % W).astype(np.int64)                  # [T, S]
    m = np.zeros((T, W), np.float32)
    np.add.at(m, (np.arange(T)[:, None], mod), 1.0)
    m += 1.0
    Cm = np.zeros((N, N), np.float32)
    rows = np.arange(T)
    for w in range(W):
        Cm[rows + w, rows] = m[:, w]

    pos = np.arange(N)
    counts = (np.minimum(pos, N - W) - np.maximum(pos - W + 1, 0) + 1)

    # padded versions for uniform band slicing
    XTp = np.zeros((B, C, N + 2 * 64), np.float16)
    for b in range(B):
        XTp[b, :, 64:64 + N] = x[b].T.astype(np.float16)
    Cmp = np.zeros((N + 2 * 64, N + 2 * 64), np.float32)
    Cmp[64:64 + N, 64:64 + N] = Cm

    Wq32 = np.asarray(Wq, np.float32)
    Wk32 = np.asarray(Wk, np.float32)
    G = Wq32.T @ Wk32                       # S = x_q G x_band^T
    g_p = _pack128(G.astype(np.float16))
    wvt_p = _pack128(np.asarray(Wv, np.float32).T.astype(np.float16))
    # g = Wk^T bq; kb_full[padded j] = fp16(x_pad)[j] . g
    g = (np.asarray(Wk, np.float32).T @ np.asarray(bq, np.float32))
    kb_full = np.zeros((B, N + 2 * 64), np.float32)
    for b in range(B):
        kb_full[b] = XTp[b].astype(np.float32).T @ g

    in_maps = []
    for core in range(NCORES):
        b, rc = divmod(core, NCORES // B)
        r0 = rc * RCH
        xt = XTp[b, :, r0:r0 + BAND]
        cmb = np.ascontiguousarray(Cmp[r0:r0 + BAND, r0:r0 + BAND])
        # all-zero columns (padded t) would give Z=0 -> 1/0*mask = NaN on
        # device; a diagonal 1 keeps Z finite there and is masked out of W
        zero_cols = ~cmb.any(axis=0)
        cmb[zero_cols, zero_cols] = 1.0
        tl = np.arange(BAND)
        rl = np.arange(RCH)
        tg = r0 - 64 + tl
        rg = r0 + rl
        d = rg[None, :] - tg[:, None]
        valid = (d >= 0) & (d <= W - 1) & (tg[:, None] >= 0) & (tg[:, None] <= T - 1)
        maskw = np.where(
            valid, 1.0 / counts[rg][None, :], 0.0
        ).astype(np.float16)
        kbias = (kb_full[b, r0:r0 + BAND] + ESHIFT).astype(np.float32)

        blob = np.zeros((128, F16), np.float16)
        blob[:, OFF_XT:OFF_XT + KC * BAND] = _pack128(xt)
        blob[:, OFF_G:OFF_G + KC * C] = g_p
        blob[:, OFF_WVT:OFF_WVT + KC * C] = wvt_p
        blob[:, OFF_MW:OFF_MW + JC * RCH] = _pack128(maskw)
        blobv = blob.view(np.uint16)
        blobv[:, OFF_KB:OFF_KB + 4] = (
            kbias.reshape(JC, 128).T.copy().view(np.uint16))

        cmu8 = np.zeros((128, NCM), np.uint8)
        cmu8[:, 0:JC * BAND] = _pack128(cmb.astype(np.uint8))
        cmu8[:, JC * BAND:] = _pack128(
            np.ascontiguousarray(cmb.T).astype(np.uint8))
        in_maps.append({"blob": blob, "cmu8": cmu8})
    return in_maps


def run_on_cores(in_maps, trace=False, trace_cores=None):
    from concourse.bass_utils import run_bass_kernel_spmd

    if "nc" not in _CACHE:
        _CACHE["nc"] = _build_program()
    nc = _CACHE["nc"]
    return run_bass_kernel_spmd(
        nc, in_maps, list(range(NCORES)), trace=trace,
        trace_cores=(trace_cores or [0]) if trace else None,
    )


def kernel(image_features, Wq, bq, Wk, bk, Wv, bv, sample_idx):
    in_maps = _host_prep(image_features, Wq, bq, Wk, bk, Wv, bv, sample_idx)
    res = run_on_cores(in_maps, trace=False)
    bv32 = np.asarray(bv, np.float32)[None, :]
    out = np.empty((B, N, C), np.float32)
    for core in range(NCORES):
        b, rc = divmod(core, NCORES // B)
        out[b, rc * RCH:(rc + 1) * RCH, :] = (
            res.results[core]["out"].astype(np.float32) + bv32)
    return out
